# revision 20
# baseline (speedup 1.0000x reference)
"""Gated Linear Attention (GLA) layer on 8 TRN2 NeuronCores.

Model: B=2, S=4096, D=2048, H=16 heads, DK=DV=128.
  q = (x@Wq)/sqrt(DK); k = x@Wk; v = x@Wv
  gk = log_sigmoid(x@Wg)/16
  S_t = diag(exp(gk_t)) S_{t-1} + k_t v_t^T;  o_t = q_t S_t
  out = o @ Wo

Sharding: core c handles batch b=c//4, head-group hg=c%4 (4 heads, 512 dims).
Each core computes its partial out = o_hg @ Wo[rows hg]; host sums the 4
partials per batch (the "all-reduce after Wo" done host-side).

Chunked-parallel recurrence (chunk C=128, inclusive in-chunk cumsum b of gk):
  qt = q*exp(b), kt = k*exp(-b)  (relative decay within chunk)
  A^T = tril-mask( kt qt^T );  o^T = V^T A^T + S_enter^T qt^T
  S_exit = diag(exp(b_C)) (S_enter + kt^T V)

Schedule/layout decisions (HW-profiled; baseline 659.7us):
- In steady state the PE never idles (weaved schedule below); the HW cost
  is PE-busy (~587us incl the chip's power-throttle derate) + startup +
  tail. This revision attacks startup/tail + engine power:
  * ALL inputs are pre-cast AND pre-packed on the host into the exact
    SBUF layouts ([part, kt, ...] packing), so every DMA is cast-free
    (HWDGE-eligible: sync/scalar queues too, not just gpsimd SWDGE) and
    reads contiguous 2-16KB lines per partition.
  * Startup weight/x0 DMAs alternate between the scalar and sync HWDGE
    queues in PE-consumption order (Wg, x0, Wq, Wk, Wv, Wo), pieces of 4
    k-tiles, so the PE chases the DMA stream instead of waiting ~50us.
  * PE warmup: ~40 dummy 128-free matmuls on a memset tile during the
    initial DMA window ramp the PE clock (0.65->2.4GHz takes ~3us of
    continuous work) so real matmuls start at speed.
  * out partials are written bf16 (halves output HBM bytes; adds ~1.1e-3
    rms to a 6.7e-3 baseline error - measured in numpy sim).
  * last chunk: each 512-col Wo slice DMAs out as soon as its PSUM
    drains, hiding the final output transfer behind the flush matmuls.
- x ships pre-transposed/pre-packed per chunk, so per-chunk xT tiles DMA
  as ONE contiguous [128, 2048] transfer into the [d, tok] layout the PE
  lhsT needs.
- Per iteration ci: stage A(ci) (projections g,q,k,v + gate path) with the
  PREVIOUS chunk's four Wo output slices interleaved into the PE stream at
  the points where PE would otherwise stall on PSUM-slot reuse or the ACT
  gate chain. The recurrence R(ci) is built as thunks and WOVEN one group
  per k-tile into the NEXT iteration's g/q projections, so its small
  matmuls' cross-engine drain waits hide behind 213ns projection matmuls
  (order is safe: wo slices consuming oT(ci) are emitted after the weave).
- o is produced directly transposed (o^T = V^T A^T + S^T q^T): skips a PE
  transpose per head and leaves o^T ready as the Wo lhsT.
- q_ps drains to SBUF with a plain copy so its PSUM bank frees before the k
  projection needs it (the decay multiply happens later from SBUF).
- The cumsum matmul runs as two bf16 matmuls on a hi/lo split of s (16-bit
  effective mantissa), half the cost of the fp32 matmul.
- exp(b_C) per head-dim = row C-1 of ebq, extracted with 1-column matmuls
  (woven into the k projection); sqrt(DK) folds into the state update.
- PSUM drains balance across DVE (qt/kt mults, qT/kT, A-mask, oT, state
  add/scale) and ACT (gate exps, q_raw/v/out drains, S_bf recast); GPSIMD
  cannot touch PSUM on TRN2.
- g projection runs in fp8 DoubleRow (2 k-tiles per instruction): gate
  errors pass through the bounded log-sigmoid slope. Host pre-scales Wg
  by 32 (into e4m3's normal range); the exp scale of -1/32 compensates.

Matmuls in bf16 (measured 2.4e-3 max rel err per K=2048 dot on HW); the gate
exp path stays fp32 (hi/lo split for the cumsum) since cumsum amplifies
rounding. Full-model max rel err vs fp32 reference: ~6.7e-3.
"""
import numpy as np

B, S, D = 2, 4096, 2048
H, DK = 16, 128
HG = 4            # head-groups (cores per batch)
HPG = H // HG     # heads per group = 4
DG = HPG * DK     # 512 dims per group
C = 128           # time chunk
NCH = S // C      # 32 chunks
NKT = D // 128    # 16 k-tiles for projections
GATE_NORM = 16.0


def _split_waits(nc, mybir, cap=1):
    """Walrus codegen rejects >1 sync wait on some instruction structs
    (fused-LDW matmul, Drain). Move excess waits onto preceding single-wait
    NOPs on the same engine — engines are in-order so this is equivalent."""
    cnt = 0

    def fix_block(b):
        nonlocal cnt
        out = []
        changed = False
        for inst in list(b.instructions):
            si = getattr(inst, "sync_info", None)
            if si is not None and len(si.on_wait) > cap:
                waits = list(si.on_wait)
                for w in waits[:-cap]:
                    nop = mybir.InstNoOp(
                        name=f"I-swait-{cnt}", ins=[], outs=[], engine=inst.engine,
                        sync_info=mybir.SyncInfo(on_wait=[w], on_update=[]))
                    cnt += 1
                    out.append(nop)
                inst.sync_info = mybir.SyncInfo(
                    on_wait=waits[-cap:], on_update=list(si.on_update))
                changed = True
            out.append(inst)
        if changed:
            b.instructions = out

    def walk(b):
        fix_block(b)
        for sb in getattr(b, "blocks", []):
            walk(sb)

    for b in nc.m.functions[0].blocks:
        walk(b)


def _build():
    import concourse.bass as bass
    import concourse.mybir as mybir
    import concourse.tile as tile

    f32 = mybir.dt.float32
    bf16 = mybir.dt.bfloat16
    fp8 = mybir.dt.float8e4
    AF = mybir.ActivationFunctionType
    MUL = mybir.AluOpType.mult
    ADD = mybir.AluOpType.add

    nc = bass.Bass()
    # Everything pre-cast AND pre-packed on the host into the SBUF layout:
    # cast-free DMAs (any-queue eligible) of contiguous per-partition lines.
    # x/xf8: [128, NCH*NKT*C]: (p, ci, kt, t) packing; chunk ci is one
    # contiguous [128, 2048] slab (4KB bf16 / 2KB fp8 lines).
    x = nc.dram_tensor("x", [128, NCH * NKT * C], bf16, kind="ExternalInput")
    xf8 = nc.dram_tensor("xf8", [128, NCH * NKT * C], fp8, kind="ExternalInput")
    # W{q,k,v}: [128, NKT*DG] = (p, kt, m);  Wg same in fp8, pre-scaled x32
    Wq = nc.dram_tensor("Wq", [128, NKT * DG], bf16, kind="ExternalInput")
    Wk = nc.dram_tensor("Wk", [128, NKT * DG], bf16, kind="ExternalInput")
    Wv = nc.dram_tensor("Wv", [128, NKT * DG], bf16, kind="ExternalInput")
    Wg = nc.dram_tensor("Wg", [128, NKT * DG], fp8, kind="ExternalInput")
    # Wo: [128, ns*HPG*512] = (p, ns, h, m) so each 512-col out slice (ns)
    # is one contiguous piece
    Wo = nc.dram_tensor("Wo", [128, HPG * D], bf16, kind="ExternalInput")
    Mc = nc.dram_tensor("Mc", [C, C], bf16, kind="ExternalInput")   # -1/16 upper-tri
    Mt = nc.dram_tensor("Mt", [C, C], f32, kind="ExternalInput")    # tril-keep mask
    el = nc.dram_tensor("el", [128, 1], f32, kind="ExternalInput")  # one-hot C-1
    out = nc.dram_tensor("out", [S, D], bf16, kind="ExternalOutput")

    with tile.TileContext(nc) as tc:
        with tc.tile_pool(name="const", bufs=1) as cpool, \
             tc.tile_pool(name="sb", bufs=2) as sb, \
             tc.tile_pool(name="st", bufs=1) as st, \
             tc.tile_pool(name="ps_t", bufs=2, space="PSUM") as ps_t, \
             tc.tile_pool(name="ps_big", bufs=3, space="PSUM") as ps_big, \
             tc.tile_pool(name="ps_rec", bufs=3, space="PSUM") as ps_rec:

            # ---- PE warmup: ramp the clock while DMAs stream ----
            warm_sb = cpool.tile([128, 128], bf16, name="warm")
            nc.vector.memset(warm_sb, 0.0)
            for wi in range(16):
                wps = ps_t.tile([128, 128], f32, name="warm_ps", tag="t")
                nc.tensor.matmul(wps, warm_sb, warm_sb, start=True, stop=True)

            # ---- weights/x0-x3/consts on the scalar/sync HWDGE queues ----
            # The tile framework's lazy sem allocator hands out ~10 unique
            # DMA semaphores, then REUSES them — and a DMA reusing a sem
            # waits for the previous user's transfer to COMPLETE before its
            # descriptor issues, serializing the queue (measured ~4us per
            # link). So: the ~10 critical-path DMAs (Wg, x0, Wq, Wk) are
            # emitted FIRST in program order (unique sems, zero waits,
            # back-to-back streaming); late-demand loads (Wv, Wo, x2, x3,
            # consts) live with reuse chains. Each piece gets its own tile
            # (two DMAs into one tile also chain on the tile's sem).
            qA, qB = nc.scalar, nc.sync
            w_tiles = {}

            def piece(name, wt, dt, i, eng):
                lo, hi = i * 8, (i + 1) * 8
                wtile = cpool.tile([128, 8 * DG], dt, name=f"W{name}{i}")
                eng.dma_start(out=wtile, in_=wt[:, lo * DG:hi * DG])
                w_tiles.setdefault(name, [None, None])[i] = wtile
                return wtile

            piece("g", Wg, fp8, 0, qA)
            x0_f8 = cpool.tile([128, D], fp8, name="x0_f8")
            qB.dma_start(out=x0_f8, in_=xf8[:, 0:D])
            piece("g", Wg, fp8, 1, qA)
            x0_bf = cpool.tile([128, D], bf16, name="x0_bf")
            qB.dma_start(out=x0_bf, in_=x[:, 0:D])
            piece("q", Wq, bf16, 0, qA)
            piece("k", Wk, bf16, 0, qB)
            piece("q", Wq, bf16, 1, qA)
            piece("k", Wk, bf16, 1, qB)
            xb1 = sb.tile([128, D], bf16, name="x_bf", bufs=3)
            qB.dma_start(out=xb1, in_=x[:, D:2 * D])
            xb81 = sb.tile([128, D], fp8, name="x_f8", bufs=3)
            qB.dma_start(out=xb81, in_=xf8[:, D:2 * D])
            piece("v", Wv, bf16, 0, qA)
            piece("v", Wv, bf16, 1, qB)
            piece("o", Wo, bf16, 0, qA)   # (p, ns, h, m): piece = ns pair
            piece("o", Wo, bf16, 1, qB)
            M_bf = cpool.tile([C, C], bf16)   # -1/16 exact in bf16
            qB.dma_start(out=M_bf, in_=Mc[:, :])
            Mt_sb = cpool.tile([C, C], f32)
            qB.dma_start(out=Mt_sb, in_=Mt[:, :])
            e_last = cpool.tile([128, 1], f32)   # one-hot selector for row C-1
            qB.dma_start(out=e_last, in_=el[:, :])
            xb2 = sb.tile([128, D], bf16, name="x_bf", bufs=3)
            qA.dma_start(out=xb2, in_=x[:, 2 * D:3 * D])
            xb82 = sb.tile([128, D], fp8, name="x_f8", bufs=3)
            qB.dma_start(out=xb82, in_=xf8[:, 2 * D:3 * D])
            xb3 = sb.tile([128, D], bf16, name="x_bf", bufs=3)
            qA.dma_start(out=xb3, in_=x[:, 3 * D:4 * D])
            xb83 = sb.tile([128, D], fp8, name="x_f8", bufs=3)
            qB.dma_start(out=xb83, in_=xf8[:, 3 * D:4 * D])
            Wo_t = w_tiles["o"]

            # ---- per-head recurrent state (fp32 master) ----
            S_st = [st.tile([DK, DK], f32, name=f"S{h}") for h in range(HPG)]
            for h in range(HPG):
                nc.vector.memset(S_st[h], 0.0)

            ln_qscale = float(np.log(DK ** -0.5))
            lnq_bias = cpool.tile([128, 1], f32)
            nc.vector.memset(lnq_bias, ln_qscale)

            def project_g(xT8, weave=None):
                # fp8 DoubleRow: one instruction contracts TWO k-tiles
                # (3D APs [p, 2, free]) at 0.5 cycles/row -> ~1.5x bf16 rate
                ps = ps_big.tile([128, DG], f32, name="ps_g", tag="big")
                for kt in range(0, NKT, 2):
                    wtile = w_tiles["g"][kt // 8]
                    co = (kt % 8) * DG
                    nc.tensor.matmul(
                        ps,
                        xT8[:, kt * 128:(kt + 2) * 128].rearrange(
                            "p (n t) -> p n t", n=2),
                        wtile[:, co:co + 2 * DG].rearrange(
                            "p (n m) -> p n m", n=2),
                        start=(kt == 0), stop=(kt == NKT - 2),
                        perf_mode=mybir.MatmulPerfMode.DoubleRow)
                    if weave and kt >= 2:
                        weave.pop(0)()
                return ps

            def project(name, xT, weave=None, weave_from=2):
                # weave: thunk list consumed one-per-k-tile (from tile 2 on);
                # each thunk emits a small recurrence group whose PSUM-drain
                # waits are then hidden behind the 213ns projection matmuls
                ps = ps_big.tile([128, DG], f32, name=f"ps_{name}", tag="big")
                for kt in range(NKT):
                    wtile = w_tiles[name][kt // 8]
                    co = (kt % 8) * DG
                    nc.tensor.matmul(
                        ps, xT[:, kt * 128:(kt + 1) * 128],
                        wtile[:, co:co + DG],
                        start=(kt == 0), stop=(kt == NKT - 1))
                    if weave and kt >= weave_from:
                        weave.pop(0)()
                return ps

            # previous-iteration tiles for the interleaved Wo stage
            oT_prev = None
            out_sb_prev = None
            t0_prev = None

            # bf16 shadow of the state for the q@S matmul; recast on ACT at
            # each iteration's tail, right after the state update
            S_bf = []
            for h in range(HPG):
                sbf = sb.tile([DK, DK], bf16, name=f"S_bf{h}")
                nc.scalar.copy(sbf, S_st[h])
                S_bf.append(sbf)

            # ---- R stage thunks ----
            # qT/kT come from XBAR DMA transposes (sync HWDGE queue) issued
            # right after qt_all/kt_all land — zero PE cycles, zero DVE
            # drains; the a/o consumers run >4us later (woven into the NEXT
            # chunk's g/q projections), far beyond the ~3us transpose chain.
            def make_rec_thunks(qT_all, kT_all, kt_all, v_bf, ebC, oT_sb,
                                update_state=True):
                A_sb = [None] * HPG

                def emit_a(h):
                    hs = slice(h * 128, (h + 1) * 128)
                    # A^T[j,i] = sum_d kt[j,d] qt[i,d]; mask keeps j<=i
                    at_ps = ps_rec.tile([C, C], f32, name="at_ps",
                                        tag="rec")
                    nc.tensor.matmul(at_ps, kT_all[:, hs], qT_all[:, hs],
                                     start=True, stop=True)
                    A_sb[h] = sb.tile([C, C], bf16, name=f"A_sb{h}")
                    nc.vector.tensor_tensor(out=A_sb[h], in0=at_ps,
                                            in1=Mt_sb, op=MUL)

                def emit_o(h):
                    hs = slice(h * 128, (h + 1) * 128)
                    # o^T = V^T A^T + S_enter^T q^T (directly transposed)
                    oT_ps = ps_rec.tile([DK, C], f32, name="oT_ps",
                                        tag="rec")
                    nc.tensor.matmul(oT_ps, v_bf[:, hs], A_sb[h],
                                     start=True, stop=False)
                    nc.tensor.matmul(oT_ps, S_bf[h], qT_all[:, hs],
                                     start=False, stop=True)
                    nc.vector.tensor_copy(oT_sb[:, hs], oT_ps)
                    if not update_state:   # last chunk: state is dead
                        return
                    # state update: S = exp(b_C)*sqrt(DK) * (S + kt^T V)
                    st_ps = ps_rec.tile([DK, DK], f32, name="st_ps",
                                        tag="rec")
                    nc.tensor.matmul(st_ps, kt_all[:, hs], v_bf[:, hs],
                                     start=True, stop=True)
                    nc.vector.tensor_tensor(out=S_st[h], in0=S_st[h],
                                            in1=st_ps, op=ADD)
                    nc.vector.tensor_scalar(out=S_st[h], in0=S_st[h],
                                            scalar1=ebC[:, h:h + 1],
                                            scalar2=float(np.sqrt(DK)),
                                            op0=MUL, op1=MUL)
                    # recast the bf16 shadow for the next chunk's q@S
                    sbf = sb.tile([DK, DK], bf16, name=f"S_bf{h}")
                    nc.scalar.copy(sbf, S_st[h])
                    S_bf[h] = sbf

                def th(f, *a):
                    return lambda: f(*a)

                # order widens the a(h) -> o(h) distance (the mask drain
                # needs ~300ns); the LAST-ITERATION flush slices below index
                # into this list: o0=3, o1=5, o2=6, o3=7 — keep in sync!
                ao_th = [
                    th(emit_a, 0), th(emit_a, 1), th(emit_a, 2),
                    th(emit_o, 0), th(emit_a, 3), th(emit_o, 1),
                    th(emit_o, 2), th(emit_o, 3),
                ]
                return ao_th

            x_bufs = [(x0_bf, x0_f8), (xb1, xb81), (xb2, xb82), (xb3, xb83)]
            rec_thunks = []   # R(ci-1) groups, woven into g-proj(ci)

            for ci in range(NCH + 1):
                last = ci == NCH

                # ---- W(ci-1) slice emitter (interleaved into A's PE stream)
                def wo_slice(ns):
                    if oT_prev is None:
                        return
                    op_ps = ps_big.tile([128, 512], f32, name=f"op{ns}",
                                        tag="big")
                    for h in range(HPG):
                        nc.tensor.matmul(
                            op_ps, oT_prev[:, h * 128:(h + 1) * 128],
                            Wo_t[ns // 2][:, (ns % 2) * 2048 +
                                          h * 512:(ns % 2) * 2048 +
                                          (h + 1) * 512],
                            start=(h == 0), stop=(h == HPG - 1))
                    nc.scalar.copy(
                        out_sb_prev[:, ns * 512:(ns + 1) * 512], op_ps)

                if last:
                    # flush R(NCH-1) with the Wo per-head matmuls interleaved
                    # as each head's oT lands (3 accumulation groups open at
                    # once - ps_big has 3 slots; slice 3 follows after).
                    # Each slice DMAs out the moment its copy lands.
                    ops = {}

                    def wo_h(ns, h):
                        if ns not in ops:
                            ops[ns] = ps_big.tile([128, 512], f32,
                                                  name=f"op{ns}", tag="big")
                        nc.tensor.matmul(
                            ops[ns], oT_prev[:, h * 128:(h + 1) * 128],
                            Wo_t[ns // 2][:, (ns % 2) * 2048 + h * 512:
                                          (ns % 2) * 2048 + (h + 1) * 512],
                            start=(h == 0), stop=(h == HPG - 1))
                        if h == HPG - 1:
                            nc.scalar.copy(
                                out_sb_prev[:, ns * 512:(ns + 1) * 512],
                                ops[ns])
                            nc.sync.dma_start(
                                out=out[t0_prev:t0_prev + 128,
                                        ns * 512:(ns + 1) * 512],
                                in_=out_sb_prev[:, ns * 512:(ns + 1) * 512])

                    for th in rec_thunks[0:4]:   # a0,a1,a2,o0
                        th()
                    for ns in range(3):
                        wo_h(ns, 0)
                    for th in rec_thunks[4:6]:   # a3,o1
                        th()
                    for ns in range(3):
                        wo_h(ns, 1)
                    rec_thunks[6]()              # o2
                    for ns in range(3):
                        wo_h(ns, 2)
                    rec_thunks[7]()              # o3
                    for ns in range(3):
                        wo_h(ns, 3)
                    # slice 3 after the rest (ps_big has 3 slots)
                    op3 = ps_big.tile([128, 512], f32, name="op3", tag="big")
                    for h in range(HPG):
                        nc.tensor.matmul(
                            op3, oT_prev[:, h * 128:(h + 1) * 128],
                            Wo_t[1][:, 2048 + h * 512:2048 + (h + 1) * 512],
                            start=(h == 0), stop=(h == HPG - 1))
                    nc.scalar.copy(out_sb_prev[:, 3 * 512:4 * 512], op3)
                    nc.sync.dma_start(
                        out=out[t0_prev:t0_prev + 128, 3 * 512:4 * 512],
                        in_=out_sb_prev[:, 3 * 512:4 * 512])
                    break

                t0 = ci * C
                xT, xT8 = x_bufs[ci]
                if 3 < ci + 1 < NCH:
                    base = (ci + 1) * D
                    xb = sb.tile([128, D], bf16, name="x_bf", bufs=3)
                    nc.gpsimd.dma_start(out=xb, in_=x[:, base:base + D])
                    xb8 = sb.tile([128, D], fp8, name="x_f8", bufs=3)
                    nc.gpsimd.dma_start(out=xb8, in_=xf8[:, base:base + D])
                    x_bufs.append((xb, xb8))

                out_sb = sb.tile([128, D], bf16, name="out_sb")

                # ---- A stage: projections (R(ci-1) woven into g+q), gate --
                g_ps = project_g(xT8, weave=rec_thunks)
                # s = softplus(-z) = ln(1 + exp(-z)); hi/lo split emitted
                # right away so the b matmuls never wait on the ACT queue
                emz = sb.tile([128, DG], f32, name="emz")
                nc.scalar.activation(emz, g_ps, AF.Exp, scale=-1.0 / 32.0)
                s_all = sb.tile([128, DG], f32, name="s_all")
                nc.scalar.activation(s_all, emz, AF.Ln, bias=1.0)
                s_hi = sb.tile([128, DG], bf16, name="s_hi")
                nc.scalar.copy(s_hi, s_all)
                s_lo = sb.tile([128, DG], bf16, name="s_lo")
                nc.vector.tensor_tensor(out=s_lo, in0=s_all, in1=s_hi,
                                        op=mybir.AluOpType.subtract)

                # q projection runs on PE while ACT finishes the gate chain;
                # the R(ci-1) leftovers finish inside it (before wo0, which
                # needs the complete oT). q_ps drains to SBUF immediately
                # (plain copy, no ebq dependency) so its PSUM slot frees
                # before the k projection needs it.
                q_ps = project("q", xT, weave=rec_thunks)
                for th in rec_thunks:   # leftovers
                    th()
                rec_thunks = []
                q_raw = sb.tile([128, DG], f32, name="q_raw")
                nc.scalar.copy(q_raw, q_ps)

                wo_slice(0)

                # in-chunk inclusive cumsum b (tokens on partitions), as two
                # bf16 matmuls over a hi/lo split of s (16-bit mantissa keeps
                # the cumsum accurate at half the fp32 matmul cost)
                b_ps = ps_big.tile([128, DG], f32, name="b_ps", tag="big")
                nc.tensor.matmul(b_ps, M_bf, s_hi, start=True, stop=False)
                nc.tensor.matmul(b_ps, M_bf, s_lo, start=False, stop=True)
                ebq = sb.tile([128, DG], f32, name="ebq")   # exp(b)/sqrt(DK)
                nc.scalar.activation(ebq, b_ps, AF.Exp, bias=lnq_bias)
                enb = sb.tile([128, DG], f32, name="enb")   # exp(-b)
                nc.scalar.activation(enb, b_ps, AF.Exp, scale=-1.0)
                qt_all = sb.tile([128, DG], bf16, name="qt_all")
                nc.vector.tensor_tensor(out=qt_all, in0=q_raw, in1=ebq, op=MUL)
                # ONE 3D XBAR transpose: qT_all[p, h, t] = qt_all[t, h*128+p]
                qT_all = sb.tile([128, DG], bf16, name="qT_all")
                nc.sync.dma_start(
                    out=qT_all.rearrange("p (h t) -> p h t", h=HPG),
                    in_=qt_all, transpose=True)

                wo_slice(1)

                # exp(b_C)/sqrt(DK) per head-dim = row C-1 of ebq, extracted
                # with 1-column matmuls woven into the k projection;
                # sqrt(DK) folds into the state scale
                ebC = sb.tile([128, HPG], f32, name="ebC")

                def tc_thunk(h):
                    def go():
                        tc_ps = ps_t.tile([128, 1], f32, name="tp_c", tag="t")
                        nc.tensor.matmul(tc_ps, ebq[:, h * 128:(h + 1) * 128],
                                         e_last, start=True, stop=True)
                        nc.scalar.copy(ebC[:, h:h + 1], tc_ps)
                    return go
                tc_thunks = [tc_thunk(h) for h in range(HPG)]

                k_ps = project("k", xT, weave=tc_thunks, weave_from=6)
                kt_all = sb.tile([128, DG], bf16, name="kt_all")
                nc.vector.tensor_tensor(out=kt_all, in0=k_ps, in1=enb, op=MUL)
                kT_all = sb.tile([128, DG], bf16, name="kT_all")
                nc.sync.dma_start(
                    out=kT_all.rearrange("p (h t) -> p h t", h=HPG),
                    in_=kt_all, transpose=True)

                wo_slice(2)

                # the a/o groups (needing the qT/kT XBAR transposes) weave
                # into g+q(ci+1)
                v_bf = sb.tile([128, DG], bf16, name="v_bf")
                oT_sb = sb.tile([128, DG], bf16, name="oT_sb")
                ao_th = make_rec_thunks(qT_all, kT_all, kt_all, v_bf, ebC,
                                        oT_sb,
                                        update_state=(ci < NCH - 1))
                v_ps = project("v", xT)
                nc.scalar.copy(v_bf, v_ps)

                wo_slice(3)
                if out_sb_prev is not None:
                    nc.sync.dma_start(out=out[t0_prev:t0_prev + 128, :],
                                      in_=out_sb_prev)

                rec_thunks = ao_th

                oT_prev = oT_sb
                out_sb_prev = out_sb
                t0_prev = t0

    _split_waits(nc, mybir)
    return nc


_NC_CACHE = None


def kernel(x, Wq, Wk, Wv, Wg, Wo):
    global _NC_CACHE
    from concourse import bass_utils
    import ml_dtypes

    bf16 = ml_dtypes.bfloat16
    fp8 = ml_dtypes.float8_e4m3fn

    x = np.asarray(x, dtype=np.float32)
    Ws = [np.asarray(w, dtype=np.float32) for w in (Wq, Wk, Wv, Wg)]
    Wo = np.asarray(Wo, dtype=np.float32)

    Mc = np.triu(np.full((C, C), -1.0 / GATE_NORM, np.float32)).astype(bf16)
    Mt = np.triu(np.ones((C, C), np.float32))          # keep j<=i
    el = np.zeros((128, 1), np.float32)
    el[C - 1, 0] = 1.0

    # x pack: [p, ci, kt, t] so each chunk is one contiguous [128, 2048]
    xp = [x[b].reshape(NCH, C, NKT, 128).transpose(3, 0, 2, 1)
          .reshape(128, NCH * NKT * C) for b in range(B)]
    xbf = [t.astype(bf16) for t in xp]
    xf8 = [t.astype(fp8) for t in xp]

    def pack_w(w):   # [2048, 512] -> [p, kt, m] = [128, NKT*DG]
        return np.ascontiguousarray(
            w.reshape(NKT, 128, DG).transpose(1, 0, 2).reshape(128, NKT * DG))

    in_maps = []
    for c in range(8):
        b, hg = c // 4, c % 4
        cols = slice(hg * DG, (hg + 1) * DG)
        # Wo slice [512, 2048] -> [p, ns, h, m(512)]
        wo_s = Wo[cols, :].reshape(HPG, 128, 4, 512).transpose(
            1, 2, 0, 3).reshape(128, HPG * D)
        m = {"x": xbf[b], "xf8": xf8[b],
             "Wo": np.ascontiguousarray(wo_s).astype(bf16),
             "Mc": Mc, "Mt": Mt, "el": el}
        for name, w in zip(("Wq", "Wk", "Wv"), Ws[:3]):
            m[name] = pack_w(w[:, cols]).astype(bf16)
        m["Wg"] = pack_w(Ws[3][:, cols] * 32.0).astype(fp8)
        in_maps.append(m)

    if _NC_CACHE is None:
        _NC_CACHE = _build()
    r = bass_utils.run_bass_kernel_spmd(_NC_CACHE, in_maps, core_ids=list(range(8)))
    globals()["_LAST_RESULTS"] = r

    res = np.zeros((B, S, D), dtype=np.float32)
    for c in range(8):
        res[c // 4] += r.results[c]["out"].astype(np.float32)
    return res


# revision 24
# speedup vs baseline: 1.0068x; 1.0068x over previous
"""Gated Linear Attention (GLA) layer on 8 TRN2 NeuronCores.

Model: B=2, S=4096, D=2048, H=16 heads, DK=DV=128.
  q = (x@Wq)/sqrt(DK); k = x@Wk; v = x@Wv
  gk = log_sigmoid(x@Wg)/16
  S_t = diag(exp(gk_t)) S_{t-1} + k_t v_t^T;  o_t = q_t S_t
  out = o @ Wo

Sharding: core c handles batch b=c//4, head-group hg=c%4 (4 heads, 512 dims).
Each core computes its partial out = o_hg @ Wo[rows hg]; host sums the 4
partials per batch (the "all-reduce after Wo" done host-side).

Chunked-parallel recurrence (chunk C=128, inclusive in-chunk cumsum b of gk):
  qt = q*exp(b), kt = k*exp(-b)  (relative decay within chunk)
  A^T = tril-mask( kt qt^T );  o^T = V^T A^T + S_enter^T qt^T
  S_exit = diag(exp(b_C)) (S_enter + kt^T V)

Schedule/layout decisions (HW-profiled; baseline 659.7us):
- In steady state the PE never idles (weaved schedule below); the HW cost
  is PE-busy (~587us incl the chip's power-throttle derate) + startup +
  tail. This revision attacks startup/tail + engine power:
  * ALL inputs are pre-cast AND pre-packed on the host into the exact
    SBUF layouts ([part, kt, ...] packing), so every DMA is cast-free
    (HWDGE-eligible: sync/scalar queues too, not just gpsimd SWDGE) and
    reads contiguous 2-16KB lines per partition.
  * Startup weight/x0 DMAs alternate between the scalar and sync HWDGE
    queues in PE-consumption order (Wg, x0, Wq, Wk, Wv, Wo), pieces of 4
    k-tiles, so the PE chases the DMA stream instead of waiting ~50us.
  * PE warmup: ~40 dummy 128-free matmuls on a memset tile during the
    initial DMA window ramp the PE clock (0.65->2.4GHz takes ~3us of
    continuous work) so real matmuls start at speed.
  * out partials are written bf16 (halves output HBM bytes; adds ~1.1e-3
    rms to a 6.7e-3 baseline error - measured in numpy sim).
  * last chunk: each 512-col Wo slice DMAs out as soon as its PSUM
    drains, hiding the final output transfer behind the flush matmuls.
- x ships pre-transposed/pre-packed per chunk, so per-chunk xT tiles DMA
  as ONE contiguous [128, 2048] transfer into the [d, tok] layout the PE
  lhsT needs.
- Per iteration ci: stage A(ci) (projections g,q,k,v + gate path) with the
  PREVIOUS chunk's four Wo output slices interleaved into the PE stream at
  the points where PE would otherwise stall on PSUM-slot reuse or the ACT
  gate chain. The recurrence R(ci) is built as thunks and WOVEN one group
  per k-tile into the NEXT iteration's g/q projections, so its small
  matmuls' cross-engine drain waits hide behind 213ns projection matmuls
  (order is safe: wo slices consuming oT(ci) are emitted after the weave).
- o is produced directly transposed (o^T = V^T A^T + S^T q^T): skips a PE
  transpose per head and leaves o^T ready as the Wo lhsT.
- q_ps drains to SBUF with a plain copy so its PSUM bank frees before the k
  projection needs it (the decay multiply happens later from SBUF).
- The cumsum matmul runs as two bf16 matmuls on a hi/lo split of s (16-bit
  effective mantissa), half the cost of the fp32 matmul.
- exp(b_C) per head-dim = row C-1 of ebq, extracted with 1-column matmuls
  (woven into the k projection); sqrt(DK) folds into the state update.
- PSUM drains balance across DVE (qt/kt mults, qT/kT, A-mask, oT, state
  add/scale) and ACT (gate exps, q_raw/v/out drains, S_bf recast); GPSIMD
  cannot touch PSUM on TRN2.
- g projection runs in fp8 DoubleRow (2 k-tiles per instruction): gate
  errors pass through the bounded log-sigmoid slope. Host pre-scales Wg
  by 32 (into e4m3's normal range); the exp scale of -1/32 compensates.

Matmuls in bf16 (measured 2.4e-3 max rel err per K=2048 dot on HW); the gate
exp path stays fp32 (hi/lo split for the cumsum) since cumsum amplifies
rounding. Full-model max rel err vs fp32 reference: ~6.7e-3.
"""
import numpy as np

B, S, D = 2, 4096, 2048
H, DK = 16, 128
HG = 4            # head-groups (cores per batch)
HPG = H // HG     # heads per group = 4
DG = HPG * DK     # 512 dims per group
C = 128           # time chunk
NCH = S // C      # 32 chunks
NKT = D // 128    # 16 k-tiles for projections
GATE_NORM = 16.0


def _split_waits(nc, mybir, cap=1):
    """Walrus codegen rejects >1 sync wait on some instruction structs
    (fused-LDW matmul, Drain). Move excess waits onto preceding single-wait
    NOPs on the same engine — engines are in-order so this is equivalent."""
    cnt = 0

    def fix_block(b):
        nonlocal cnt
        out = []
        changed = False
        for inst in list(b.instructions):
            si = getattr(inst, "sync_info", None)
            if si is not None and len(si.on_wait) > cap:
                waits = list(si.on_wait)
                for w in waits[:-cap]:
                    nop = mybir.InstNoOp(
                        name=f"I-swait-{cnt}", ins=[], outs=[], engine=inst.engine,
                        sync_info=mybir.SyncInfo(on_wait=[w], on_update=[]))
                    cnt += 1
                    out.append(nop)
                inst.sync_info = mybir.SyncInfo(
                    on_wait=waits[-cap:], on_update=list(si.on_update))
                changed = True
            out.append(inst)
        if changed:
            b.instructions = out

    def walk(b):
        fix_block(b)
        for sb in getattr(b, "blocks", []):
            walk(sb)

    for b in nc.m.functions[0].blocks:
        walk(b)


def _build():
    import concourse.bass as bass
    import concourse.mybir as mybir
    import concourse.tile as tile

    f32 = mybir.dt.float32
    bf16 = mybir.dt.bfloat16
    fp8 = mybir.dt.float8e4
    AF = mybir.ActivationFunctionType
    MUL = mybir.AluOpType.mult
    ADD = mybir.AluOpType.add

    nc = bass.Bass()
    # Everything pre-cast AND pre-packed on the host into the SBUF layout:
    # cast-free DMAs (any-queue eligible) of contiguous per-partition lines.
    # x/xf8: [128, NCH*NKT*C]: (p, ci, kt, t) packing; chunk ci is one
    # contiguous [128, 2048] slab (4KB bf16 / 2KB fp8 lines).
    x = nc.dram_tensor("x", [128, NCH * NKT * C], bf16, kind="ExternalInput")
    xf8 = nc.dram_tensor("xf8", [128, NCH * NKT * C], fp8, kind="ExternalInput")
    # W{q,k,v}: [128, NKT*DG] = (p, kt, m);  Wg same in fp8, pre-scaled x32
    Wq = nc.dram_tensor("Wq", [128, NKT * DG], bf16, kind="ExternalInput")
    Wk = nc.dram_tensor("Wk", [128, NKT * DG], bf16, kind="ExternalInput")
    Wv = nc.dram_tensor("Wv", [128, NKT * DG], bf16, kind="ExternalInput")
    Wg = nc.dram_tensor("Wg", [128, NKT * DG], fp8, kind="ExternalInput")
    # Wo: [128, ns*HPG*512] = (p, ns, h, m) so each 512-col out slice (ns)
    # is one contiguous piece
    Wo = nc.dram_tensor("Wo", [128, HPG * D], bf16, kind="ExternalInput")
    Mc = nc.dram_tensor("Mc", [C, C], bf16, kind="ExternalInput")   # -1/16 upper-tri
    Mt = nc.dram_tensor("Mt", [C, C], f32, kind="ExternalInput")    # tril-keep mask
    el = nc.dram_tensor("el", [128, 1], f32, kind="ExternalInput")  # one-hot C-1
    out = nc.dram_tensor("out", [S, D], bf16, kind="ExternalOutput")

    with tile.TileContext(nc) as tc:
        with tc.tile_pool(name="const", bufs=1) as cpool, \
             tc.tile_pool(name="sb", bufs=2) as sb, \
             tc.tile_pool(name="st", bufs=1) as st, \
             tc.tile_pool(name="ps_t", bufs=2, space="PSUM") as ps_t, \
             tc.tile_pool(name="ps_big", bufs=3, space="PSUM") as ps_big, \
             tc.tile_pool(name="ps_rec", bufs=3, space="PSUM") as ps_rec:

            # ---- PE warmup: ramp the clock while DMAs stream ----
            warm_sb = cpool.tile([128, 128], bf16, name="warm")
            nc.vector.memset(warm_sb, 0.0)
            for wi in range(28):
                wps = ps_t.tile([128, 128], f32, name="warm_ps", tag="t")
                nc.tensor.matmul(wps, warm_sb, warm_sb, start=True, stop=True)

            # ---- weights/x0-x3/consts on the scalar/sync HWDGE queues ----
            # The tile framework's lazy sem allocator hands out ~10 unique
            # DMA semaphores, then REUSES them — and a DMA reusing a sem
            # waits for the previous user's transfer to COMPLETE before its
            # descriptor issues, serializing the queue (measured ~4us per
            # link). So: the ~10 critical-path DMAs (Wg, x0, Wq, Wk) are
            # emitted FIRST in program order (unique sems, zero waits,
            # back-to-back streaming); late-demand loads (Wv, Wo, x2, x3,
            # consts) live with reuse chains. Each piece gets its own tile
            # (two DMAs into one tile also chain on the tile's sem).
            qA, qB = nc.scalar, nc.sync
            w_tiles = {}

            def piece(name, wt, dt, i, eng):
                lo, hi = i * 8, (i + 1) * 8
                wtile = cpool.tile([128, 8 * DG], dt, name=f"W{name}{i}")
                eng.dma_start(out=wtile, in_=wt[:, lo * DG:hi * DG])
                w_tiles.setdefault(name, [None, None])[i] = wtile
                return wtile

            piece("g", Wg, fp8, 0, qA)
            x0_f8 = cpool.tile([128, D], fp8, name="x0_f8")
            qB.dma_start(out=x0_f8, in_=xf8[:, 0:D])
            piece("g", Wg, fp8, 1, qA)
            x0_bf = cpool.tile([128, D], bf16, name="x0_bf")
            qB.dma_start(out=x0_bf, in_=x[:, 0:D])
            piece("q", Wq, bf16, 0, qA)
            piece("k", Wk, bf16, 0, qB)
            piece("q", Wq, bf16, 1, qA)
            piece("k", Wk, bf16, 1, qB)
            xb1 = sb.tile([128, D], bf16, name="x_bf", bufs=3)
            qB.dma_start(out=xb1, in_=x[:, D:2 * D])
            xb81 = sb.tile([128, D], fp8, name="x_f8", bufs=3)
            qB.dma_start(out=xb81, in_=xf8[:, D:2 * D])
            piece("v", Wv, bf16, 0, qA)
            piece("v", Wv, bf16, 1, qB)
            piece("o", Wo, bf16, 0, qA)   # (p, ns, h, m): piece = ns pair
            piece("o", Wo, bf16, 1, qB)
            M_bf = cpool.tile([C, C], bf16)   # -1/16 exact in bf16
            qB.dma_start(out=M_bf, in_=Mc[:, :])
            Mt_sb = cpool.tile([C, C], f32)
            qB.dma_start(out=Mt_sb, in_=Mt[:, :])
            e_last = cpool.tile([128, 1], f32)   # one-hot selector for row C-1
            qB.dma_start(out=e_last, in_=el[:, :])
            Wo_t = w_tiles["o"]

            # ---- per-head recurrent state (fp32 master) ----
            S_st = [st.tile([DK, DK], f32, name=f"S{h}") for h in range(HPG)]
            for h in range(HPG):
                nc.vector.memset(S_st[h], 0.0)

            ln_qscale = float(np.log(DK ** -0.5))
            lnq_bias = cpool.tile([128, 1], f32)
            nc.vector.memset(lnq_bias, ln_qscale)

            def project_g(xT8, weave=None):
                # fp8 DoubleRow: one instruction contracts TWO k-tiles
                # (3D APs [p, 2, free]) at 0.5 cycles/row -> ~1.5x bf16 rate
                ps = ps_big.tile([128, DG], f32, name="ps_g", tag="big")
                for kt in range(0, NKT, 2):
                    wtile = w_tiles["g"][kt // 8]
                    co = (kt % 8) * DG
                    nc.tensor.matmul(
                        ps,
                        xT8[:, kt * 128:(kt + 2) * 128].rearrange(
                            "p (n t) -> p n t", n=2),
                        wtile[:, co:co + 2 * DG].rearrange(
                            "p (n m) -> p n m", n=2),
                        start=(kt == 0), stop=(kt == NKT - 2),
                        perf_mode=mybir.MatmulPerfMode.DoubleRow)
                    if weave and kt >= 2:
                        weave.pop(0)()
                return ps

            def project(name, xT, weave=None, weave_from=2):
                # weave: thunk list consumed one-per-k-tile (from tile 2 on);
                # each thunk emits a small recurrence group whose PSUM-drain
                # waits are then hidden behind the 213ns projection matmuls
                ps = ps_big.tile([128, DG], f32, name=f"ps_{name}", tag="big")
                for kt in range(NKT):
                    wtile = w_tiles[name][kt // 8]
                    co = (kt % 8) * DG
                    nc.tensor.matmul(
                        ps, xT[:, kt * 128:(kt + 1) * 128],
                        wtile[:, co:co + DG],
                        start=(kt == 0), stop=(kt == NKT - 1))
                    if weave and kt >= weave_from:
                        weave.pop(0)()
                return ps

            # previous-iteration tiles for the interleaved Wo stage
            oT_prev = None
            out_sb_prev = None
            t0_prev = None

            # bf16 shadow of the state for the q@S matmul; recast on ACT at
            # each iteration's tail, right after the state update
            S_bf = []
            for h in range(HPG):
                sbf = sb.tile([DK, DK], bf16, name=f"S_bf{h}")
                nc.scalar.copy(sbf, S_st[h])
                S_bf.append(sbf)

            # ---- R stage thunks ----
            # qT/kT come from XBAR DMA transposes (sync HWDGE queue) issued
            # right after qt_all/kt_all land — zero PE cycles, zero DVE
            # drains; the a/o consumers run >4us later (woven into the NEXT
            # chunk's g/q projections), far beyond the ~3us transpose chain.
            def make_rec_thunks(qT_all, kT_all, kt_all, v_bf, ebC, oT_sb,
                                update_state=True):
                A_sb = [None] * HPG

                def emit_a(h):
                    hs = slice(h * 128, (h + 1) * 128)
                    # A^T[j,i] = sum_d kt[j,d] qt[i,d]; mask keeps j<=i
                    at_ps = ps_rec.tile([C, C], f32, name="at_ps",
                                        tag="rec")
                    nc.tensor.matmul(at_ps, kT_all[:, hs], qT_all[:, hs],
                                     start=True, stop=True)
                    A_sb[h] = sb.tile([C, C], bf16, name=f"A_sb{h}")
                    nc.vector.tensor_tensor(out=A_sb[h], in0=at_ps,
                                            in1=Mt_sb, op=MUL)

                def emit_o(h):
                    hs = slice(h * 128, (h + 1) * 128)
                    # o^T = V^T A^T + S_enter^T q^T (directly transposed)
                    oT_ps = ps_rec.tile([DK, C], f32, name="oT_ps",
                                        tag="rec")
                    nc.tensor.matmul(oT_ps, v_bf[:, hs], A_sb[h],
                                     start=True, stop=False)
                    nc.tensor.matmul(oT_ps, S_bf[h], qT_all[:, hs],
                                     start=False, stop=True)
                    nc.vector.tensor_copy(oT_sb[:, hs], oT_ps)
                    if not update_state:   # last chunk: state is dead
                        return
                    # state update: S = exp(b_C)*sqrt(DK) * (S + kt^T V)
                    st_ps = ps_rec.tile([DK, DK], f32, name="st_ps",
                                        tag="rec")
                    nc.tensor.matmul(st_ps, kt_all[:, hs], v_bf[:, hs],
                                     start=True, stop=True)
                    nc.vector.tensor_tensor(out=S_st[h], in0=S_st[h],
                                            in1=st_ps, op=ADD)
                    nc.vector.tensor_scalar(out=S_st[h], in0=S_st[h],
                                            scalar1=ebC[:, h:h + 1],
                                            scalar2=float(np.sqrt(DK)),
                                            op0=MUL, op1=MUL)
                    # recast the bf16 shadow for the next chunk's q@S
                    sbf = sb.tile([DK, DK], bf16, name=f"S_bf{h}")
                    nc.scalar.copy(sbf, S_st[h])
                    S_bf[h] = sbf

                def th(f, *a):
                    return lambda: f(*a)

                # order widens the a(h) -> o(h) distance (the mask drain
                # needs ~300ns); the LAST-ITERATION flush slices below index
                # into this list: o0=3, o1=5, o2=6, o3=7 — keep in sync!
                ao_th = [
                    th(emit_a, 0), th(emit_a, 1), th(emit_a, 2),
                    th(emit_o, 0), th(emit_a, 3), th(emit_o, 1),
                    th(emit_o, 2), th(emit_o, 3),
                ]
                return ao_th

            x_bufs = [(x0_bf, x0_f8), (xb1, xb81)]
            rec_thunks = []   # R(ci-1) groups, woven into g-proj(ci)

            for ci in range(NCH + 1):
                last = ci == NCH

                # ---- W(ci-1) slice emitter (interleaved into A's PE stream)
                def wo_slice(ns):
                    if oT_prev is None:
                        return
                    op_ps = ps_big.tile([128, 512], f32, name=f"op{ns}",
                                        tag="big")
                    for h in range(HPG):
                        nc.tensor.matmul(
                            op_ps, oT_prev[:, h * 128:(h + 1) * 128],
                            Wo_t[ns // 2][:, (ns % 2) * 2048 +
                                          h * 512:(ns % 2) * 2048 +
                                          (h + 1) * 512],
                            start=(h == 0), stop=(h == HPG - 1))
                    nc.scalar.copy(
                        out_sb_prev[:, ns * 512:(ns + 1) * 512], op_ps)

                if last:
                    # flush R(NCH-1) with the Wo per-head matmuls interleaved
                    # as each head's oT lands (3 accumulation groups open at
                    # once - ps_big has 3 slots; slice 3 follows after).
                    # Each slice DMAs out the moment its copy lands.
                    ops = {}

                    def wo_h(ns, h):
                        if ns not in ops:
                            ops[ns] = ps_big.tile([128, 512], f32,
                                                  name=f"op{ns}", tag="big")
                        nc.tensor.matmul(
                            ops[ns], oT_prev[:, h * 128:(h + 1) * 128],
                            Wo_t[ns // 2][:, (ns % 2) * 2048 + h * 512:
                                          (ns % 2) * 2048 + (h + 1) * 512],
                            start=(h == 0), stop=(h == HPG - 1))
                        if h == HPG - 1:
                            nc.scalar.copy(
                                out_sb_prev[:, ns * 512:(ns + 1) * 512],
                                ops[ns])
                            nc.sync.dma_start(
                                out=out[t0_prev:t0_prev + 128,
                                        ns * 512:(ns + 1) * 512],
                                in_=out_sb_prev[:, ns * 512:(ns + 1) * 512])

                    for th in rec_thunks[0:4]:   # a0,a1,a2,o0
                        th()
                    for ns in range(3):
                        wo_h(ns, 0)
                    for th in rec_thunks[4:6]:   # a3,o1
                        th()
                    for ns in range(3):
                        wo_h(ns, 1)
                    rec_thunks[6]()              # o2
                    for ns in range(3):
                        wo_h(ns, 2)
                    rec_thunks[7]()              # o3
                    for ns in range(3):
                        wo_h(ns, 3)
                    # slice 3 after the rest (ps_big has 3 slots)
                    op3 = ps_big.tile([128, 512], f32, name="op3", tag="big")
                    for h in range(HPG):
                        nc.tensor.matmul(
                            op3, oT_prev[:, h * 128:(h + 1) * 128],
                            Wo_t[1][:, 2048 + h * 512:2048 + (h + 1) * 512],
                            start=(h == 0), stop=(h == HPG - 1))
                    nc.scalar.copy(out_sb_prev[:, 3 * 512:4 * 512], op3)
                    nc.sync.dma_start(
                        out=out[t0_prev:t0_prev + 128, 3 * 512:4 * 512],
                        in_=out_sb_prev[:, 3 * 512:4 * 512])
                    break

                t0 = ci * C
                xT, xT8 = x_bufs[ci]
                if 1 <= ci and ci + 1 < NCH:
                    # x2/x3 ride the HWDGE queues (FIFO after all setup DMAs,
                    # so they don't steal HBM from the weight stream); x4+ on
                    # gpsimd, naturally gated by pool-slot reuse (the slot's
                    # previous chunk must be consumed first).
                    base = (ci + 1) * D
                    xb = sb.tile([128, D], bf16, name="x_bf", bufs=3)
                    xb8 = sb.tile([128, D], fp8, name="x_f8", bufs=3)
                    if ci + 1 <= 3:
                        qA.dma_start(out=xb, in_=x[:, base:base + D])
                        qB.dma_start(out=xb8, in_=xf8[:, base:base + D])
                    else:
                        nc.gpsimd.dma_start(out=xb, in_=x[:, base:base + D])
                        nc.gpsimd.dma_start(out=xb8, in_=xf8[:, base:base + D])
                    x_bufs.append((xb, xb8))

                out_sb = sb.tile([128, D], bf16, name="out_sb")

                # ---- A stage: projections (R(ci-1) woven into g+q), gate --
                g_ps = project_g(xT8, weave=rec_thunks)
                # s = softplus(-z) = ln(1 + exp(-z)); hi/lo split emitted
                # right away so the b matmuls never wait on the ACT queue
                emz = sb.tile([128, DG], f32, name="emz")
                nc.scalar.activation(emz, g_ps, AF.Exp, scale=-1.0 / 32.0)
                s_all = sb.tile([128, DG], f32, name="s_all")
                nc.scalar.activation(s_all, emz, AF.Ln, bias=1.0)
                s_hi = sb.tile([128, DG], bf16, name="s_hi")
                nc.scalar.copy(s_hi, s_all)
                s_lo = sb.tile([128, DG], bf16, name="s_lo")
                nc.vector.tensor_tensor(out=s_lo, in0=s_all, in1=s_hi,
                                        op=mybir.AluOpType.subtract)

                # q projection runs on PE while ACT finishes the gate chain;
                # the R(ci-1) leftovers finish inside it (before wo0, which
                # needs the complete oT). q_ps drains to SBUF immediately
                # (plain copy, no ebq dependency) so its PSUM slot frees
                # before the k projection needs it.
                q_ps = project("q", xT, weave=rec_thunks)
                for th in rec_thunks:   # leftovers
                    th()
                rec_thunks = []
                q_raw = sb.tile([128, DG], f32, name="q_raw")
                nc.scalar.copy(q_raw, q_ps)

                wo_slice(0)

                # in-chunk inclusive cumsum b (tokens on partitions), as two
                # bf16 matmuls over a hi/lo split of s (16-bit mantissa keeps
                # the cumsum accurate at half the fp32 matmul cost)
                b_ps = ps_big.tile([128, DG], f32, name="b_ps", tag="big")
                nc.tensor.matmul(b_ps, M_bf, s_hi, start=True, stop=False)
                nc.tensor.matmul(b_ps, M_bf, s_lo, start=False, stop=True)
                ebq = sb.tile([128, DG], f32, name="ebq")   # exp(b)/sqrt(DK)
                nc.scalar.activation(ebq, b_ps, AF.Exp, bias=lnq_bias)
                enb = sb.tile([128, DG], f32, name="enb")   # exp(-b)
                nc.scalar.activation(enb, b_ps, AF.Exp, scale=-1.0)
                qt_all = sb.tile([128, DG], bf16, name="qt_all")
                nc.vector.tensor_tensor(out=qt_all, in0=q_raw, in1=ebq, op=MUL)
                # ONE 3D XBAR transpose: qT_all[p, h, t] = qt_all[t, h*128+p]
                qT_all = sb.tile([128, DG], bf16, name="qT_all")
                nc.sync.dma_start(
                    out=qT_all.rearrange("p (h t) -> p h t", h=HPG),
                    in_=qt_all, transpose=True)

                wo_slice(1)

                # exp(b_C)/sqrt(DK) per head-dim = row C-1 of ebq, extracted
                # with 1-column matmuls woven into the k projection;
                # sqrt(DK) folds into the state scale
                ebC = sb.tile([128, HPG], f32, name="ebC")

                def tc_thunk(h):
                    def go():
                        tc_ps = ps_t.tile([128, 1], f32, name="tp_c", tag="t")
                        nc.tensor.matmul(tc_ps, ebq[:, h * 128:(h + 1) * 128],
                                         e_last, start=True, stop=True)
                        nc.scalar.copy(ebC[:, h:h + 1], tc_ps)
                    return go
                tc_thunks = [tc_thunk(h) for h in range(HPG)]

                k_ps = project("k", xT, weave=tc_thunks, weave_from=6)
                kt_all = sb.tile([128, DG], bf16, name="kt_all")
                nc.vector.tensor_tensor(out=kt_all, in0=k_ps, in1=enb, op=MUL)
                kT_all = sb.tile([128, DG], bf16, name="kT_all")
                nc.sync.dma_start(
                    out=kT_all.rearrange("p (h t) -> p h t", h=HPG),
                    in_=kt_all, transpose=True)

                wo_slice(2)

                # the a/o groups (needing the qT/kT XBAR transposes) weave
                # into g+q(ci+1)
                v_bf = sb.tile([128, DG], bf16, name="v_bf")
                oT_sb = sb.tile([128, DG], bf16, name="oT_sb")
                ao_th = make_rec_thunks(qT_all, kT_all, kt_all, v_bf, ebC,
                                        oT_sb,
                                        update_state=(ci < NCH - 1))
                v_ps = project("v", xT)
                nc.scalar.copy(v_bf, v_ps)

                wo_slice(3)
                if out_sb_prev is not None:
                    nc.sync.dma_start(out=out[t0_prev:t0_prev + 128, :],
                                      in_=out_sb_prev)

                rec_thunks = ao_th

                oT_prev = oT_sb
                out_sb_prev = out_sb
                t0_prev = t0

    _split_waits(nc, mybir)
    return nc


_NC_CACHE = None


def kernel(x, Wq, Wk, Wv, Wg, Wo):
    global _NC_CACHE
    from concourse import bass_utils
    import ml_dtypes

    bf16 = ml_dtypes.bfloat16
    fp8 = ml_dtypes.float8_e4m3fn

    x = np.asarray(x, dtype=np.float32)
    Ws = [np.asarray(w, dtype=np.float32) for w in (Wq, Wk, Wv, Wg)]
    Wo = np.asarray(Wo, dtype=np.float32)

    Mc = np.triu(np.full((C, C), -1.0 / GATE_NORM, np.float32)).astype(bf16)
    Mt = np.triu(np.ones((C, C), np.float32))          # keep j<=i
    el = np.zeros((128, 1), np.float32)
    el[C - 1, 0] = 1.0

    # x pack: [p, ci, kt, t] so each chunk is one contiguous [128, 2048]
    xp = [x[b].reshape(NCH, C, NKT, 128).transpose(3, 0, 2, 1)
          .reshape(128, NCH * NKT * C) for b in range(B)]
    xbf = [t.astype(bf16) for t in xp]
    xf8 = [t.astype(fp8) for t in xp]

    def pack_w(w):   # [2048, 512] -> [p, kt, m] = [128, NKT*DG]
        return np.ascontiguousarray(
            w.reshape(NKT, 128, DG).transpose(1, 0, 2).reshape(128, NKT * DG))

    in_maps = []
    for c in range(8):
        b, hg = c // 4, c % 4
        cols = slice(hg * DG, (hg + 1) * DG)
        # Wo slice [512, 2048] -> [p, ns, h, m(512)]
        wo_s = Wo[cols, :].reshape(HPG, 128, 4, 512).transpose(
            1, 2, 0, 3).reshape(128, HPG * D)
        m = {"x": xbf[b], "xf8": xf8[b],
             "Wo": np.ascontiguousarray(wo_s).astype(bf16),
             "Mc": Mc, "Mt": Mt, "el": el}
        for name, w in zip(("Wq", "Wk", "Wv"), Ws[:3]):
            m[name] = pack_w(w[:, cols]).astype(bf16)
        m["Wg"] = pack_w(Ws[3][:, cols] * 32.0).astype(fp8)
        in_maps.append(m)

    if _NC_CACHE is None:
        _NC_CACHE = _build()
    r = bass_utils.run_bass_kernel_spmd(_NC_CACHE, in_maps, core_ids=list(range(8)))
    globals()["_LAST_RESULTS"] = r

    res = np.zeros((B, S, D), dtype=np.float32)
    for c in range(8):
        res[c // 4] += r.results[c]["out"].astype(np.float32)
    return res


# revision 25
# speedup vs baseline: 1.0279x; 1.0209x over previous
"""Gated Linear Attention (GLA) layer on 8 TRN2 NeuronCores.

Model: B=2, S=4096, D=2048, H=16 heads, DK=DV=128.
  q = (x@Wq)/sqrt(DK); k = x@Wk; v = x@Wv
  gk = log_sigmoid(x@Wg)/16
  S_t = diag(exp(gk_t)) S_{t-1} + k_t v_t^T;  o_t = q_t S_t
  out = o @ Wo

Sharding: core c handles batch b=c//4, head-group hg=c%4 (4 heads, 512 dims).
Each core computes its partial out = o_hg @ Wo[rows hg]; host sums the 4
partials per batch (the "all-reduce after Wo" done host-side).

Chunked-parallel recurrence (chunk C=128, inclusive in-chunk cumsum b of gk):
  qt = q*exp(b), kt = k*exp(-b)  (relative decay within chunk)
  A^T = tril-mask( kt qt^T );  o^T = V^T A^T + S_enter^T qt^T
  S_exit = diag(exp(b_C)) (S_enter + kt^T V)

Schedule/layout decisions (HW-profiled; baseline 659.7us):
- In steady state the PE never idles (weaved schedule below); the HW cost
  is PE-busy (~587us incl the chip's power-throttle derate) + startup +
  tail. This revision attacks startup/tail + engine power:
  * ALL inputs are pre-cast AND pre-packed on the host into the exact
    SBUF layouts ([part, kt, ...] packing), so every DMA is cast-free
    (HWDGE-eligible: sync/scalar queues too, not just gpsimd SWDGE) and
    reads contiguous 2-16KB lines per partition.
  * Startup weight/x0 DMAs alternate between the scalar and sync HWDGE
    queues in PE-consumption order (Wg, x0, Wq, Wk, Wv, Wo), pieces of 4
    k-tiles, so the PE chases the DMA stream instead of waiting ~50us.
  * PE warmup: ~40 dummy 128-free matmuls on a memset tile during the
    initial DMA window ramp the PE clock (0.65->2.4GHz takes ~3us of
    continuous work) so real matmuls start at speed.
  * out partials are written bf16 (halves output HBM bytes; adds ~1.1e-3
    rms to a 6.7e-3 baseline error - measured in numpy sim).
  * last chunk: each 512-col Wo slice DMAs out as soon as its PSUM
    drains, hiding the final output transfer behind the flush matmuls.
- x ships pre-transposed/pre-packed per chunk, so per-chunk xT tiles DMA
  as ONE contiguous [128, 2048] transfer into the [d, tok] layout the PE
  lhsT needs.
- Per iteration ci: stage A(ci) (projections g,q,k,v + gate path) with the
  PREVIOUS chunk's four Wo output slices interleaved into the PE stream at
  the points where PE would otherwise stall on PSUM-slot reuse or the ACT
  gate chain. The recurrence R(ci) is built as thunks and WOVEN one group
  per k-tile into the NEXT iteration's g/q projections, so its small
  matmuls' cross-engine drain waits hide behind 213ns projection matmuls
  (order is safe: wo slices consuming oT(ci) are emitted after the weave).
- o is produced directly transposed (o^T = V^T A^T + S^T q^T): skips a PE
  transpose per head and leaves o^T ready as the Wo lhsT.
- q_ps drains to SBUF with a plain copy so its PSUM bank frees before the k
  projection needs it (the decay multiply happens later from SBUF).
- The cumsum matmul runs as two bf16 matmuls on a hi/lo split of s (16-bit
  effective mantissa), half the cost of the fp32 matmul.
- exp(b_C) per head-dim = row C-1 of ebq, extracted with 1-column matmuls
  (woven into the k projection); sqrt(DK) folds into the state update.
- PSUM drains balance across DVE (qt/kt mults, qT/kT, A-mask, oT, state
  add/scale) and ACT (gate exps, q_raw/v/out drains, S_bf recast); GPSIMD
  cannot touch PSUM on TRN2.
- g projection runs in fp8 DoubleRow (2 k-tiles per instruction): gate
  errors pass through the bounded log-sigmoid slope. Host pre-scales Wg
  by 32 (into e4m3's normal range); the exp scale of -1/32 compensates.

Matmuls in bf16 (measured 2.4e-3 max rel err per K=2048 dot on HW); the gate
exp path stays fp32 (hi/lo split for the cumsum) since cumsum amplifies
rounding. Full-model max rel err vs fp32 reference: ~6.7e-3.
"""
import numpy as np

B, S, D = 2, 4096, 2048
H, DK = 16, 128
HG = 4            # head-groups (cores per batch)
HPG = H // HG     # heads per group = 4
DG = HPG * DK     # 512 dims per group
C = 128           # time chunk
NCH = S // C      # 32 chunks
NKT = D // 128    # 16 k-tiles for projections
GATE_NORM = 16.0


def _split_waits(nc, mybir, cap=1):
    """Walrus codegen rejects >1 sync wait on some instruction structs
    (fused-LDW matmul, Drain). Move excess waits onto preceding single-wait
    NOPs on the same engine — engines are in-order so this is equivalent."""
    cnt = 0

    def fix_block(b):
        nonlocal cnt
        out = []
        changed = False
        for inst in list(b.instructions):
            si = getattr(inst, "sync_info", None)
            if si is not None and len(si.on_wait) > cap:
                waits = list(si.on_wait)
                for w in waits[:-cap]:
                    nop = mybir.InstNoOp(
                        name=f"I-swait-{cnt}", ins=[], outs=[], engine=inst.engine,
                        sync_info=mybir.SyncInfo(on_wait=[w], on_update=[]))
                    cnt += 1
                    out.append(nop)
                inst.sync_info = mybir.SyncInfo(
                    on_wait=waits[-cap:], on_update=list(si.on_update))
                changed = True
            out.append(inst)
        if changed:
            b.instructions = out

    def walk(b):
        fix_block(b)
        for sb in getattr(b, "blocks", []):
            walk(sb)

    for b in nc.m.functions[0].blocks:
        walk(b)


def _build():
    import concourse.bass as bass
    import concourse.mybir as mybir
    import concourse.tile as tile

    f32 = mybir.dt.float32
    bf16 = mybir.dt.bfloat16
    fp8 = mybir.dt.float8e4
    AF = mybir.ActivationFunctionType
    MUL = mybir.AluOpType.mult
    ADD = mybir.AluOpType.add

    nc = bass.Bass()
    # Everything pre-cast AND pre-packed on the host into the SBUF layout:
    # cast-free DMAs (any-queue eligible) of contiguous per-partition lines.
    # x/xf8: [128, NCH*NKT*C]: (p, ci, kt, t) packing; chunk ci is one
    # contiguous [128, 2048] slab (4KB bf16 / 2KB fp8 lines).
    x = nc.dram_tensor("x", [128, NCH * NKT * C], bf16, kind="ExternalInput")
    xf8 = nc.dram_tensor("xf8", [128, NCH * NKT * C], fp8, kind="ExternalInput")
    # W{q,k,v}: [128, NKT*DG] = (p, kt, m);  Wg same in fp8, pre-scaled x32
    Wq = nc.dram_tensor("Wq", [128, NKT * DG], bf16, kind="ExternalInput")
    Wk = nc.dram_tensor("Wk", [128, NKT * DG], bf16, kind="ExternalInput")
    Wv = nc.dram_tensor("Wv", [128, NKT * DG], bf16, kind="ExternalInput")
    Wg = nc.dram_tensor("Wg", [128, NKT * DG], fp8, kind="ExternalInput")
    # Wo: [128, ns*HPG*512] = (p, ns, h, m) so each 512-col out slice (ns)
    # is one contiguous piece
    Wo = nc.dram_tensor("Wo", [128, HPG * D], bf16, kind="ExternalInput")
    Mc = nc.dram_tensor("Mc", [C, C], bf16, kind="ExternalInput")   # -1/16 upper-tri
    Mt = nc.dram_tensor("Mt", [C, C], f32, kind="ExternalInput")    # tril-keep mask
    el = nc.dram_tensor("el", [128, 1], bf16, kind="ExternalInput")  # one-hot C-1
    out = nc.dram_tensor("out", [S, D], bf16, kind="ExternalOutput")

    with tile.TileContext(nc) as tc:
        with tc.tile_pool(name="const", bufs=1) as cpool, \
             tc.tile_pool(name="sb", bufs=2) as sb, \
             tc.tile_pool(name="st", bufs=1) as st, \
             tc.tile_pool(name="ps_t", bufs=2, space="PSUM") as ps_t, \
             tc.tile_pool(name="ps_big", bufs=3, space="PSUM") as ps_big, \
             tc.tile_pool(name="ps_rec", bufs=3, space="PSUM") as ps_rec:

            # ---- PE warmup: ramp the clock while DMAs stream ----
            warm_sb = cpool.tile([128, 128], bf16, name="warm")
            nc.vector.memset(warm_sb, 0.0)
            for wi in range(28):
                wps = ps_t.tile([128, 128], f32, name="warm_ps", tag="t")
                nc.tensor.matmul(wps, warm_sb, warm_sb, start=True, stop=True)

            # ---- weights/x0-x3/consts on the scalar/sync HWDGE queues ----
            # The tile framework's lazy sem allocator hands out ~10 unique
            # DMA semaphores, then REUSES them — and a DMA reusing a sem
            # waits for the previous user's transfer to COMPLETE before its
            # descriptor issues, serializing the queue (measured ~4us per
            # link). So: the ~10 critical-path DMAs (Wg, x0, Wq, Wk) are
            # emitted FIRST in program order (unique sems, zero waits,
            # back-to-back streaming); late-demand loads (Wv, Wo, x2, x3,
            # consts) live with reuse chains. Each piece gets its own tile
            # (two DMAs into one tile also chain on the tile's sem).
            qA, qB = nc.scalar, nc.sync
            w_tiles = {}

            def piece(name, wt, dt, i, eng):
                lo, hi = i * 8, (i + 1) * 8
                wtile = cpool.tile([128, 8 * DG], dt, name=f"W{name}{i}")
                eng.dma_start(out=wtile, in_=wt[:, lo * DG:hi * DG])
                w_tiles.setdefault(name, [None, None])[i] = wtile
                return wtile

            piece("g", Wg, fp8, 0, qA)
            x0_f8 = cpool.tile([128, D], fp8, name="x0_f8")
            qB.dma_start(out=x0_f8, in_=xf8[:, 0:D])
            piece("g", Wg, fp8, 1, qA)
            x0_bf = cpool.tile([128, D], bf16, name="x0_bf")
            qB.dma_start(out=x0_bf, in_=x[:, 0:D])
            piece("q", Wq, bf16, 0, qA)
            piece("k", Wk, bf16, 0, qB)
            piece("q", Wq, bf16, 1, qA)
            piece("k", Wk, bf16, 1, qB)
            xb1 = sb.tile([128, D], bf16, name="x_bf", bufs=3)
            qB.dma_start(out=xb1, in_=x[:, D:2 * D])
            xb81 = sb.tile([128, D], fp8, name="x_f8", bufs=3)
            qB.dma_start(out=xb81, in_=xf8[:, D:2 * D])
            piece("v", Wv, bf16, 0, qA)
            piece("v", Wv, bf16, 1, qB)
            piece("o", Wo, bf16, 0, qA)   # (p, ns, h, m): piece = ns pair
            piece("o", Wo, bf16, 1, qB)
            M_bf = cpool.tile([C, C], bf16)   # -1/16 exact in bf16
            qB.dma_start(out=M_bf, in_=Mc[:, :])
            Mt_sb = cpool.tile([C, C], f32)
            qB.dma_start(out=Mt_sb, in_=Mt[:, :])
            e_last = cpool.tile([128, 1], bf16)  # one-hot selector for row C-1
            qB.dma_start(out=e_last, in_=el[:, :])
            Wo_t = w_tiles["o"]

            # ---- per-head recurrent state (fp32 master) ----
            S_st = [st.tile([DK, DK], f32, name=f"S{h}") for h in range(HPG)]
            for h in range(HPG):
                nc.vector.memset(S_st[h], 0.0)

            ln_qscale = float(np.log(DK ** -0.5))
            lnq_bias = cpool.tile([128, 1], f32)
            nc.vector.memset(lnq_bias, ln_qscale)

            def project_g(xT8, weave=None):
                # fp8 DoubleRow: one instruction contracts TWO k-tiles
                # (3D APs [p, 2, free]) at 0.5 cycles/row -> ~1.5x bf16 rate
                ps = ps_big.tile([128, DG], f32, name="ps_g", tag="big")
                for kt in range(0, NKT, 2):
                    wtile = w_tiles["g"][kt // 8]
                    co = (kt % 8) * DG
                    nc.tensor.matmul(
                        ps,
                        xT8[:, kt * 128:(kt + 2) * 128].rearrange(
                            "p (n t) -> p n t", n=2),
                        wtile[:, co:co + 2 * DG].rearrange(
                            "p (n m) -> p n m", n=2),
                        start=(kt == 0), stop=(kt == NKT - 2),
                        perf_mode=mybir.MatmulPerfMode.DoubleRow)
                    if weave and kt >= 2:
                        weave.pop(0)()
                return ps

            def project(name, xT, weave=None, weave_from=2):
                # weave: thunk list consumed one-per-k-tile (from tile 2 on);
                # each thunk emits a small recurrence group whose PSUM-drain
                # waits are then hidden behind the 213ns projection matmuls
                ps = ps_big.tile([128, DG], f32, name=f"ps_{name}", tag="big")
                for kt in range(NKT):
                    wtile = w_tiles[name][kt // 8]
                    co = (kt % 8) * DG
                    nc.tensor.matmul(
                        ps, xT[:, kt * 128:(kt + 1) * 128],
                        wtile[:, co:co + DG],
                        start=(kt == 0), stop=(kt == NKT - 1))
                    if weave and kt >= weave_from:
                        weave.pop(0)()
                return ps

            # previous-iteration tiles for the interleaved Wo stage
            oT_prev = None
            out_sb_prev = None
            t0_prev = None

            # bf16 shadow of the state for the q@S matmul; recast on ACT at
            # each iteration's tail, right after the state update
            S_bf = []
            for h in range(HPG):
                sbf = sb.tile([DK, DK], bf16, name=f"S_bf{h}")
                nc.scalar.copy(sbf, S_st[h])
                S_bf.append(sbf)

            # ---- R stage thunks ----
            # qT/kT come from XBAR DMA transposes (sync HWDGE queue) issued
            # right after qt_all/kt_all land — zero PE cycles, zero DVE
            # drains; the a/o consumers run >4us later (woven into the NEXT
            # chunk's g/q projections), far beyond the ~3us transpose chain.
            def make_rec_thunks(qT_all, kT_all, kt_all, v_bf, ebC, oT_sb,
                                update_state=True):
                A_sb = [None] * HPG

                def emit_a(h):
                    hs = slice(h * 128, (h + 1) * 128)
                    # A^T[j,i] = sum_d kt[j,d] qt[i,d]; mask keeps j<=i
                    at_ps = ps_rec.tile([C, C], f32, name="at_ps",
                                        tag="rec")
                    nc.tensor.matmul(at_ps, kT_all[:, hs], qT_all[:, hs],
                                     start=True, stop=True)
                    A_sb[h] = sb.tile([C, C], bf16, name=f"A_sb{h}")
                    nc.vector.tensor_tensor(out=A_sb[h], in0=at_ps,
                                            in1=Mt_sb, op=MUL)

                def emit_o(h):
                    hs = slice(h * 128, (h + 1) * 128)
                    # o^T = V^T A^T + S_enter^T q^T (directly transposed)
                    oT_ps = ps_rec.tile([DK, C], f32, name="oT_ps",
                                        tag="rec")
                    nc.tensor.matmul(oT_ps, v_bf[:, hs], A_sb[h],
                                     start=True, stop=False)
                    nc.tensor.matmul(oT_ps, S_bf[h], qT_all[:, hs],
                                     start=False, stop=True)
                    nc.vector.tensor_copy(oT_sb[:, hs], oT_ps)
                    if not update_state:   # last chunk: state is dead
                        return
                    # state update: S = exp(b_C)*sqrt(DK) * (S + kt^T V)
                    st_ps = ps_rec.tile([DK, DK], f32, name="st_ps",
                                        tag="rec")
                    nc.tensor.matmul(st_ps, kt_all[:, hs], v_bf[:, hs],
                                     start=True, stop=True)
                    nc.vector.tensor_tensor(out=S_st[h], in0=S_st[h],
                                            in1=st_ps, op=ADD)
                    nc.vector.tensor_scalar(out=S_st[h], in0=S_st[h],
                                            scalar1=ebC[:, h:h + 1],
                                            scalar2=float(np.sqrt(DK)),
                                            op0=MUL, op1=MUL)
                    # recast the bf16 shadow for the next chunk's q@S
                    sbf = sb.tile([DK, DK], bf16, name=f"S_bf{h}")
                    nc.scalar.copy(sbf, S_st[h])
                    S_bf[h] = sbf

                def th(f, *a):
                    return lambda: f(*a)

                # order widens the a(h) -> o(h) distance (the mask drain
                # needs ~300ns); the LAST-ITERATION flush slices below index
                # into this list: o0=3, o1=5, o2=6, o3=7 — keep in sync!
                ao_th = [
                    th(emit_a, 0), th(emit_a, 1), th(emit_a, 2),
                    th(emit_o, 0), th(emit_a, 3), th(emit_o, 1),
                    th(emit_o, 2), th(emit_o, 3),
                ]
                return ao_th

            x_bufs = [(x0_bf, x0_f8), (xb1, xb81)]
            rec_thunks = []   # R(ci-1) groups, woven into g-proj(ci)

            for ci in range(NCH + 1):
                last = ci == NCH

                # ---- W(ci-1) slice emitter (interleaved into A's PE stream)
                def wo_slice(ns):
                    if oT_prev is None:
                        return
                    op_ps = ps_big.tile([128, 512], f32, name=f"op{ns}",
                                        tag="big")
                    for h in range(HPG):
                        nc.tensor.matmul(
                            op_ps, oT_prev[:, h * 128:(h + 1) * 128],
                            Wo_t[ns // 2][:, (ns % 2) * 2048 +
                                          h * 512:(ns % 2) * 2048 +
                                          (h + 1) * 512],
                            start=(h == 0), stop=(h == HPG - 1))
                    nc.scalar.copy(
                        out_sb_prev[:, ns * 512:(ns + 1) * 512], op_ps)

                if last:
                    # flush R(NCH-1) with the Wo per-head matmuls interleaved
                    # as each head's oT lands (3 accumulation groups open at
                    # once - ps_big has 3 slots; slice 3 follows after).
                    # Each slice DMAs out the moment its copy lands.
                    ops = {}

                    def wo_h(ns, h):
                        if ns not in ops:
                            ops[ns] = ps_big.tile([128, 512], f32,
                                                  name=f"op{ns}", tag="big")
                        nc.tensor.matmul(
                            ops[ns], oT_prev[:, h * 128:(h + 1) * 128],
                            Wo_t[ns // 2][:, (ns % 2) * 2048 + h * 512:
                                          (ns % 2) * 2048 + (h + 1) * 512],
                            start=(h == 0), stop=(h == HPG - 1))
                        if h == HPG - 1:
                            nc.scalar.copy(
                                out_sb_prev[:, ns * 512:(ns + 1) * 512],
                                ops[ns])
                            nc.sync.dma_start(
                                out=out[t0_prev:t0_prev + 128,
                                        ns * 512:(ns + 1) * 512],
                                in_=out_sb_prev[:, ns * 512:(ns + 1) * 512])

                    for th in rec_thunks[0:4]:   # a0,a1,a2,o0
                        th()
                    for ns in range(3):
                        wo_h(ns, 0)
                    for th in rec_thunks[4:6]:   # a3,o1
                        th()
                    for ns in range(3):
                        wo_h(ns, 1)
                    rec_thunks[6]()              # o2
                    for ns in range(3):
                        wo_h(ns, 2)
                    rec_thunks[7]()              # o3
                    for ns in range(3):
                        wo_h(ns, 3)
                    # slice 3 after the rest (ps_big has 3 slots)
                    op3 = ps_big.tile([128, 512], f32, name="op3", tag="big")
                    for h in range(HPG):
                        nc.tensor.matmul(
                            op3, oT_prev[:, h * 128:(h + 1) * 128],
                            Wo_t[1][:, 2048 + h * 512:2048 + (h + 1) * 512],
                            start=(h == 0), stop=(h == HPG - 1))
                    nc.scalar.copy(out_sb_prev[:, 3 * 512:4 * 512], op3)
                    nc.sync.dma_start(
                        out=out[t0_prev:t0_prev + 128, 3 * 512:4 * 512],
                        in_=out_sb_prev[:, 3 * 512:4 * 512])
                    break

                t0 = ci * C
                xT, xT8 = x_bufs[ci]
                if 1 <= ci and ci + 1 < NCH:
                    # x2/x3 ride the HWDGE queues (FIFO after all setup DMAs,
                    # so they don't steal HBM from the weight stream); x4+ on
                    # gpsimd, naturally gated by pool-slot reuse (the slot's
                    # previous chunk must be consumed first).
                    base = (ci + 1) * D
                    xb = sb.tile([128, D], bf16, name="x_bf", bufs=3)
                    xb8 = sb.tile([128, D], fp8, name="x_f8", bufs=3)
                    if ci + 1 <= 3:
                        qA.dma_start(out=xb, in_=x[:, base:base + D])
                        qB.dma_start(out=xb8, in_=xf8[:, base:base + D])
                    else:
                        nc.gpsimd.dma_start(out=xb, in_=x[:, base:base + D])
                        nc.gpsimd.dma_start(out=xb8, in_=xf8[:, base:base + D])
                    x_bufs.append((xb, xb8))

                out_sb = sb.tile([128, D], bf16, name="out_sb")

                # ---- A stage: projections (R(ci-1) woven into g+q), gate --
                g_ps = project_g(xT8, weave=rec_thunks)
                # s = softplus(-z) = ln(1 + exp(-z)); hi/lo split emitted
                # right away so the b matmuls never wait on the ACT queue
                emz = sb.tile([128, DG], f32, name="emz")
                nc.scalar.activation(emz, g_ps, AF.Exp, scale=-1.0 / 32.0)
                s_all = sb.tile([128, DG], f32, name="s_all")
                nc.scalar.activation(s_all, emz, AF.Ln, bias=1.0)
                s_hi = sb.tile([128, DG], bf16, name="s_hi")
                nc.scalar.copy(s_hi, s_all)
                s_lo = sb.tile([128, DG], bf16, name="s_lo")
                nc.vector.tensor_tensor(out=s_lo, in0=s_all, in1=s_hi,
                                        op=mybir.AluOpType.subtract)

                # q projection runs on PE while ACT finishes the gate chain;
                # the R(ci-1) leftovers finish inside it (before wo0, which
                # needs the complete oT). q_ps drains to SBUF immediately
                # (plain copy, no ebq dependency) so its PSUM slot frees
                # before the k projection needs it.
                q_ps = project("q", xT, weave=rec_thunks)
                for th in rec_thunks:   # leftovers
                    th()
                rec_thunks = []
                q_raw = sb.tile([128, DG], f32, name="q_raw")
                nc.scalar.copy(q_raw, q_ps)

                wo_slice(0)

                # in-chunk inclusive cumsum b (tokens on partitions), as two
                # bf16 matmuls over a hi/lo split of s (16-bit mantissa keeps
                # the cumsum accurate at half the fp32 matmul cost)
                b_ps = ps_big.tile([128, DG], f32, name="b_ps", tag="big")
                nc.tensor.matmul(b_ps, M_bf, s_hi, start=True, stop=False)
                nc.tensor.matmul(b_ps, M_bf, s_lo, start=False, stop=True)
                # bf16: single-pass PE matmul for the ebC extraction (fp32
                # lhsT = 2-pass + fp32 LDW, ~0.6us/chunk); 0.2% rounding on
                # qt/state-scale is below the bf16 noise already present
                ebq = sb.tile([128, DG], bf16, name="ebq")  # exp(b)/sqrt(DK)
                nc.scalar.activation(ebq, b_ps, AF.Exp, bias=lnq_bias)
                enb = sb.tile([128, DG], bf16, name="enb")  # exp(-b)
                nc.scalar.activation(enb, b_ps, AF.Exp, scale=-1.0)
                qt_all = sb.tile([128, DG], bf16, name="qt_all")
                nc.vector.tensor_tensor(out=qt_all, in0=q_raw, in1=ebq, op=MUL)
                # ONE 3D XBAR transpose: qT_all[p, h, t] = qt_all[t, h*128+p]
                qT_all = sb.tile([128, DG], bf16, name="qT_all")
                nc.sync.dma_start(
                    out=qT_all.rearrange("p (h t) -> p h t", h=HPG),
                    in_=qt_all, transpose=True)

                wo_slice(1)

                # exp(b_C)/sqrt(DK) per head-dim = row C-1 of ebq, extracted
                # with 1-column matmuls woven into the k projection;
                # sqrt(DK) folds into the state scale
                ebC = sb.tile([128, HPG], f32, name="ebC")

                def tc_thunk(h):
                    def go():
                        tc_ps = ps_t.tile([128, 1], f32, name="tp_c", tag="t")
                        nc.tensor.matmul(tc_ps, ebq[:, h * 128:(h + 1) * 128],
                                         e_last, start=True, stop=True)
                        nc.scalar.copy(ebC[:, h:h + 1], tc_ps)
                    return go
                tc_thunks = [tc_thunk(h) for h in range(HPG)]

                k_ps = project("k", xT, weave=tc_thunks, weave_from=6)
                kt_all = sb.tile([128, DG], bf16, name="kt_all")
                nc.vector.tensor_tensor(out=kt_all, in0=k_ps, in1=enb, op=MUL)
                kT_all = sb.tile([128, DG], bf16, name="kT_all")
                nc.sync.dma_start(
                    out=kT_all.rearrange("p (h t) -> p h t", h=HPG),
                    in_=kt_all, transpose=True)

                wo_slice(2)

                # the a/o groups (needing the qT/kT XBAR transposes) weave
                # into g+q(ci+1)
                v_bf = sb.tile([128, DG], bf16, name="v_bf")
                oT_sb = sb.tile([128, DG], bf16, name="oT_sb")
                ao_th = make_rec_thunks(qT_all, kT_all, kt_all, v_bf, ebC,
                                        oT_sb,
                                        update_state=(ci < NCH - 1))
                v_ps = project("v", xT)
                nc.scalar.copy(v_bf, v_ps)

                wo_slice(3)
                if out_sb_prev is not None:
                    nc.sync.dma_start(out=out[t0_prev:t0_prev + 128, :],
                                      in_=out_sb_prev)

                rec_thunks = ao_th

                oT_prev = oT_sb
                out_sb_prev = out_sb
                t0_prev = t0

    _split_waits(nc, mybir)
    return nc


_NC_CACHE = None


def kernel(x, Wq, Wk, Wv, Wg, Wo):
    global _NC_CACHE
    from concourse import bass_utils
    import ml_dtypes

    bf16 = ml_dtypes.bfloat16
    fp8 = ml_dtypes.float8_e4m3fn

    x = np.asarray(x, dtype=np.float32)
    Ws = [np.asarray(w, dtype=np.float32) for w in (Wq, Wk, Wv, Wg)]
    Wo = np.asarray(Wo, dtype=np.float32)

    Mc = np.triu(np.full((C, C), -1.0 / GATE_NORM, np.float32)).astype(bf16)
    Mt = np.triu(np.ones((C, C), np.float32))          # keep j<=i
    el = np.zeros((128, 1), np.float32)
    el[C - 1, 0] = 1.0
    el = el.astype(bf16)

    # x pack: [p, ci, kt, t] so each chunk is one contiguous [128, 2048]
    xp = [x[b].reshape(NCH, C, NKT, 128).transpose(3, 0, 2, 1)
          .reshape(128, NCH * NKT * C) for b in range(B)]
    xbf = [t.astype(bf16) for t in xp]
    xf8 = [t.astype(fp8) for t in xp]

    def pack_w(w):   # [2048, 512] -> [p, kt, m] = [128, NKT*DG]
        return np.ascontiguousarray(
            w.reshape(NKT, 128, DG).transpose(1, 0, 2).reshape(128, NKT * DG))

    in_maps = []
    for c in range(8):
        b, hg = c // 4, c % 4
        cols = slice(hg * DG, (hg + 1) * DG)
        # Wo slice [512, 2048] -> [p, ns, h, m(512)]
        wo_s = Wo[cols, :].reshape(HPG, 128, 4, 512).transpose(
            1, 2, 0, 3).reshape(128, HPG * D)
        m = {"x": xbf[b], "xf8": xf8[b],
             "Wo": np.ascontiguousarray(wo_s).astype(bf16),
             "Mc": Mc, "Mt": Mt, "el": el}
        for name, w in zip(("Wq", "Wk", "Wv"), Ws[:3]):
            m[name] = pack_w(w[:, cols]).astype(bf16)
        m["Wg"] = pack_w(Ws[3][:, cols] * 32.0).astype(fp8)
        in_maps.append(m)

    if _NC_CACHE is None:
        _NC_CACHE = _build()
    r = bass_utils.run_bass_kernel_spmd(_NC_CACHE, in_maps, core_ids=list(range(8)))
    globals()["_LAST_RESULTS"] = r

    res = np.zeros((B, S, D), dtype=np.float32)
    for c in range(8):
        res[c // 4] += r.results[c]["out"].astype(np.float32)
    return res


# revision 26
# speedup vs baseline: 1.0367x; 1.0086x over previous
"""Gated Linear Attention (GLA) layer on 8 TRN2 NeuronCores.

Model: B=2, S=4096, D=2048, H=16 heads, DK=DV=128.
  q = (x@Wq)/sqrt(DK); k = x@Wk; v = x@Wv
  gk = log_sigmoid(x@Wg)/16
  S_t = diag(exp(gk_t)) S_{t-1} + k_t v_t^T;  o_t = q_t S_t
  out = o @ Wo

Sharding: core c handles batch b=c//4, head-group hg=c%4 (4 heads, 512 dims).
Each core computes its partial out = o_hg @ Wo[rows hg]; host sums the 4
partials per batch (the "all-reduce after Wo" done host-side).

Chunked-parallel recurrence (chunk C=128, inclusive in-chunk cumsum b of gk):
  qt = q*exp(b), kt = k*exp(-b)  (relative decay within chunk)
  A^T = tril-mask( kt qt^T );  o^T = V^T A^T + S_enter^T qt^T
  S_exit = diag(exp(b_C)) (S_enter + kt^T V)

Schedule/layout decisions (HW-profiled; baseline 659.7us):
- In steady state the PE never idles (weaved schedule below); the HW cost
  is PE-busy (~587us incl the chip's power-throttle derate) + startup +
  tail. This revision attacks startup/tail + engine power:
  * ALL inputs are pre-cast AND pre-packed on the host into the exact
    SBUF layouts ([part, kt, ...] packing), so every DMA is cast-free
    (HWDGE-eligible: sync/scalar queues too, not just gpsimd SWDGE) and
    reads contiguous 2-16KB lines per partition.
  * Startup weight/x0 DMAs alternate between the scalar and sync HWDGE
    queues in PE-consumption order (Wg, x0, Wq, Wk, Wv, Wo), pieces of 4
    k-tiles, so the PE chases the DMA stream instead of waiting ~50us.
  * PE warmup: ~40 dummy 128-free matmuls on a memset tile during the
    initial DMA window ramp the PE clock (0.65->2.4GHz takes ~3us of
    continuous work) so real matmuls start at speed.
  * out partials are written bf16 (halves output HBM bytes; adds ~1.1e-3
    rms to a 6.7e-3 baseline error - measured in numpy sim).
  * last chunk: each 512-col Wo slice DMAs out as soon as its PSUM
    drains, hiding the final output transfer behind the flush matmuls.
- x ships pre-transposed/pre-packed per chunk, so per-chunk xT tiles DMA
  as ONE contiguous [128, 2048] transfer into the [d, tok] layout the PE
  lhsT needs.
- Per iteration ci: stage A(ci) (projections g,q,k,v + gate path) with the
  PREVIOUS chunk's four Wo output slices interleaved into the PE stream at
  the points where PE would otherwise stall on PSUM-slot reuse or the ACT
  gate chain. The recurrence R(ci) is built as thunks and WOVEN one group
  per k-tile into the NEXT iteration's g/q projections, so its small
  matmuls' cross-engine drain waits hide behind 213ns projection matmuls
  (order is safe: wo slices consuming oT(ci) are emitted after the weave).
- o is produced directly transposed (o^T = V^T A^T + S^T q^T): skips a PE
  transpose per head and leaves o^T ready as the Wo lhsT.
- q_ps drains to SBUF with a plain copy so its PSUM bank frees before the k
  projection needs it (the decay multiply happens later from SBUF).
- The cumsum matmul runs as two bf16 matmuls on a hi/lo split of s (16-bit
  effective mantissa), half the cost of the fp32 matmul.
- exp(b_C) per head-dim = row C-1 of ebq, extracted with 1-column matmuls
  (woven into the k projection); sqrt(DK) folds into the state update.
- PSUM drains balance across DVE (qt/kt mults, qT/kT, A-mask, oT, state
  add/scale) and ACT (gate exps, q_raw/v/out drains, S_bf recast); GPSIMD
  cannot touch PSUM on TRN2.
- g projection runs in fp8 DoubleRow (2 k-tiles per instruction): gate
  errors pass through the bounded log-sigmoid slope. Host pre-scales Wg
  by 32 (into e4m3's normal range); the exp scale of -1/32 compensates.

Matmuls in bf16 (measured 2.4e-3 max rel err per K=2048 dot on HW); the gate
exp path stays fp32 (hi/lo split for the cumsum) since cumsum amplifies
rounding. Full-model max rel err vs fp32 reference: ~6.7e-3.
"""
import numpy as np

B, S, D = 2, 4096, 2048
H, DK = 16, 128
HG = 4            # head-groups (cores per batch)
HPG = H // HG     # heads per group = 4
DG = HPG * DK     # 512 dims per group
C = 128           # time chunk
NCH = S // C      # 32 chunks
NKT = D // 128    # 16 k-tiles for projections
GATE_NORM = 16.0


def _split_waits(nc, mybir, cap=1):
    """Walrus codegen rejects >1 sync wait on some instruction structs
    (fused-LDW matmul, Drain). Move excess waits onto preceding single-wait
    NOPs on the same engine — engines are in-order so this is equivalent."""
    cnt = 0

    def fix_block(b):
        nonlocal cnt
        out = []
        changed = False
        for inst in list(b.instructions):
            si = getattr(inst, "sync_info", None)
            if si is not None and len(si.on_wait) > cap:
                waits = list(si.on_wait)
                for w in waits[:-cap]:
                    nop = mybir.InstNoOp(
                        name=f"I-swait-{cnt}", ins=[], outs=[], engine=inst.engine,
                        sync_info=mybir.SyncInfo(on_wait=[w], on_update=[]))
                    cnt += 1
                    out.append(nop)
                inst.sync_info = mybir.SyncInfo(
                    on_wait=waits[-cap:], on_update=list(si.on_update))
                changed = True
            out.append(inst)
        if changed:
            b.instructions = out

    def walk(b):
        fix_block(b)
        for sb in getattr(b, "blocks", []):
            walk(sb)

    for b in nc.m.functions[0].blocks:
        walk(b)


def _build():
    import concourse.bass as bass
    import concourse.mybir as mybir
    import concourse.tile as tile

    f32 = mybir.dt.float32
    bf16 = mybir.dt.bfloat16
    fp8 = mybir.dt.float8e4
    AF = mybir.ActivationFunctionType
    MUL = mybir.AluOpType.mult
    ADD = mybir.AluOpType.add

    nc = bass.Bass()
    # Everything pre-cast AND pre-packed on the host into the SBUF layout:
    # cast-free DMAs (any-queue eligible) of contiguous per-partition lines.
    # x/xf8: [128, NCH*NKT*C]: (p, ci, kt, t) packing; chunk ci is one
    # contiguous [128, 2048] slab (4KB bf16 / 2KB fp8 lines).
    x = nc.dram_tensor("x", [128, NCH * NKT * C], bf16, kind="ExternalInput")
    xf8 = nc.dram_tensor("xf8", [128, NCH * NKT * C], fp8, kind="ExternalInput")
    # W{q,k,v}: [128, NKT*DG] = (p, kt, m);  Wg same in fp8, pre-scaled x32
    Wq = nc.dram_tensor("Wq", [128, NKT * DG], bf16, kind="ExternalInput")
    Wk = nc.dram_tensor("Wk", [128, NKT * DG], bf16, kind="ExternalInput")
    Wv = nc.dram_tensor("Wv", [128, NKT * DG], bf16, kind="ExternalInput")
    Wg = nc.dram_tensor("Wg", [128, NKT * DG], fp8, kind="ExternalInput")
    # Wo: [128, ns*HPG*512] = (p, ns, h, m) so each 512-col out slice (ns)
    # is one contiguous piece
    Wo = nc.dram_tensor("Wo", [128, HPG * D], bf16, kind="ExternalInput")
    Mc = nc.dram_tensor("Mc", [C, C], bf16, kind="ExternalInput")   # -1/16 upper-tri
    Mt = nc.dram_tensor("Mt", [C, C], f32, kind="ExternalInput")    # tril-keep mask
    el = nc.dram_tensor("el", [128, 1], bf16, kind="ExternalInput")  # one-hot C-1
    out = nc.dram_tensor("out", [S, D], bf16, kind="ExternalOutput")

    with tile.TileContext(nc) as tc:
        with tc.tile_pool(name="const", bufs=1) as cpool, \
             tc.tile_pool(name="sb", bufs=2) as sb, \
             tc.tile_pool(name="st", bufs=1) as st, \
             tc.tile_pool(name="ps_t", bufs=2, space="PSUM") as ps_t, \
             tc.tile_pool(name="ps_big", bufs=3, space="PSUM") as ps_big, \
             tc.tile_pool(name="ps_rec", bufs=3, space="PSUM") as ps_rec:

            # ---- PE warmup: ramp the clock while DMAs stream ----
            warm_sb = cpool.tile([128, 128], bf16, name="warm")
            nc.vector.memset(warm_sb, 0.0)
            for wi in range(28):
                wps = ps_t.tile([128, 128], f32, name="warm_ps", tag="t")
                nc.tensor.matmul(wps, warm_sb, warm_sb, start=True, stop=True)

            # ---- weights/x0-x3/consts on the scalar/sync HWDGE queues ----
            # The tile framework's lazy sem allocator hands out ~10 unique
            # DMA semaphores, then REUSES them — and a DMA reusing a sem
            # waits for the previous user's transfer to COMPLETE before its
            # descriptor issues, serializing the queue (measured ~4us per
            # link). So: the ~10 critical-path DMAs (Wg, x0, Wq, Wk) are
            # emitted FIRST in program order (unique sems, zero waits,
            # back-to-back streaming); late-demand loads (Wv, Wo, x2, x3,
            # consts) live with reuse chains. Each piece gets its own tile
            # (two DMAs into one tile also chain on the tile's sem).
            qA, qB = nc.scalar, nc.sync
            w_tiles = {}

            def piece(name, wt, dt, i, eng):
                lo, hi = i * 8, (i + 1) * 8
                wtile = cpool.tile([128, 8 * DG], dt, name=f"W{name}{i}")
                eng.dma_start(out=wtile, in_=wt[:, lo * DG:hi * DG])
                w_tiles.setdefault(name, [None, None])[i] = wtile
                return wtile

            piece("g", Wg, fp8, 0, qA)
            x0_f8 = cpool.tile([128, D], fp8, name="x0_f8")
            qB.dma_start(out=x0_f8, in_=xf8[:, 0:D])
            piece("g", Wg, fp8, 1, qA)
            x0_bf = cpool.tile([128, D], bf16, name="x0_bf")
            qB.dma_start(out=x0_bf, in_=x[:, 0:D])
            piece("q", Wq, bf16, 0, qA)
            piece("k", Wk, bf16, 0, qB)
            piece("q", Wq, bf16, 1, qA)
            piece("k", Wk, bf16, 1, qB)
            xb1 = sb.tile([128, D], bf16, name="x_bf", bufs=3)
            qB.dma_start(out=xb1, in_=x[:, D:2 * D])
            xb81 = sb.tile([128, D], fp8, name="x_f8", bufs=3)
            qB.dma_start(out=xb81, in_=xf8[:, D:2 * D])
            piece("v", Wv, bf16, 0, qA)
            piece("v", Wv, bf16, 1, qB)
            piece("o", Wo, bf16, 0, qA)   # (p, ns, h, m): piece = ns pair
            piece("o", Wo, bf16, 1, qB)
            M_bf = cpool.tile([C, C], bf16)   # -1/16 exact in bf16
            qB.dma_start(out=M_bf, in_=Mc[:, :])
            Mt_sb = cpool.tile([C, C], f32)
            qB.dma_start(out=Mt_sb, in_=Mt[:, :])
            e_last = cpool.tile([128, 1], bf16)  # one-hot selector for row C-1
            qB.dma_start(out=e_last, in_=el[:, :])
            Wo_t = w_tiles["o"]

            # ---- per-head recurrent state (fp32 master) ----
            S_st = [st.tile([DK, DK], f32, name=f"S{h}") for h in range(HPG)]
            for h in range(HPG):
                nc.vector.memset(S_st[h], 0.0)

            ln_qscale = float(np.log(DK ** -0.5))
            lnq_bias = cpool.tile([128, 1], f32)
            nc.vector.memset(lnq_bias, ln_qscale)

            def project_g(xT8, weave=None):
                # fp8 DoubleRow: one instruction contracts TWO k-tiles
                # (3D APs [p, 2, free]) at 0.5 cycles/row -> ~1.5x bf16 rate
                ps = ps_big.tile([128, DG], f32, name="ps_g", tag="big")
                for kt in range(0, NKT, 2):
                    wtile = w_tiles["g"][kt // 8]
                    co = (kt % 8) * DG
                    nc.tensor.matmul(
                        ps,
                        xT8[:, kt * 128:(kt + 2) * 128].rearrange(
                            "p (n t) -> p n t", n=2),
                        wtile[:, co:co + 2 * DG].rearrange(
                            "p (n m) -> p n m", n=2),
                        start=(kt == 0), stop=(kt == NKT - 2),
                        perf_mode=mybir.MatmulPerfMode.DoubleRow)
                    if weave and kt >= 2:
                        weave.pop(0)()
                return ps

            def project(name, xT, weave=None, weave_from=2):
                # weave: thunk list consumed one-per-k-tile (from tile 2 on);
                # each thunk emits a small recurrence group whose PSUM-drain
                # waits are then hidden behind the 213ns projection matmuls
                ps = ps_big.tile([128, DG], f32, name=f"ps_{name}", tag="big")
                for kt in range(NKT):
                    wtile = w_tiles[name][kt // 8]
                    co = (kt % 8) * DG
                    nc.tensor.matmul(
                        ps, xT[:, kt * 128:(kt + 1) * 128],
                        wtile[:, co:co + DG],
                        start=(kt == 0), stop=(kt == NKT - 1))
                    if weave and kt >= weave_from:
                        weave.pop(0)()
                return ps

            # previous-iteration tiles for the interleaved Wo stage
            oT_prev = None
            out_sb_prev = None
            t0_prev = None

            # bf16 shadow of the state for the q@S matmul; recast on ACT at
            # each iteration's tail, right after the state update
            S_bf = []
            for h in range(HPG):
                sbf = sb.tile([DK, DK], bf16, name=f"S_bf{h}")
                nc.scalar.copy(sbf, S_st[h])
                S_bf.append(sbf)

            # ---- R stage thunks ----
            # qT/kT come from XBAR DMA transposes (sync HWDGE queue) issued
            # right after qt_all/kt_all land — zero PE cycles, zero DVE
            # drains; the a/o consumers run >4us later (woven into the NEXT
            # chunk's g/q projections), far beyond the ~3us transpose chain.
            def make_rec_thunks(qT_all, kT_all, kt_all, v_bf, ebC, oT_sb,
                                update_state=True):
                A_sb = [None] * HPG

                def emit_a(h):
                    hs = slice(h * 128, (h + 1) * 128)
                    # A^T[j,i] = sum_d kt[j,d] qt[i,d]; mask keeps j<=i
                    at_ps = ps_rec.tile([C, C], f32, name="at_ps",
                                        tag="rec")
                    nc.tensor.matmul(at_ps, kT_all[:, hs], qT_all[:, hs],
                                     start=True, stop=True)
                    A_sb[h] = sb.tile([C, C], bf16, name=f"A_sb{h}")
                    nc.vector.tensor_tensor(out=A_sb[h], in0=at_ps,
                                            in1=Mt_sb, op=MUL)

                def emit_o(h):
                    hs = slice(h * 128, (h + 1) * 128)
                    # o^T = V^T A^T + S_enter^T q^T (directly transposed)
                    oT_ps = ps_rec.tile([DK, C], f32, name="oT_ps",
                                        tag="rec")
                    nc.tensor.matmul(oT_ps, v_bf[:, hs], A_sb[h],
                                     start=True, stop=False)
                    nc.tensor.matmul(oT_ps, S_bf[h], qT_all[:, hs],
                                     start=False, stop=True)
                    nc.vector.tensor_copy(oT_sb[:, hs], oT_ps)
                    if not update_state:   # last chunk: state is dead
                        return
                    # state update: S = exp(b_C)*sqrt(DK) * (S + kt^T V)
                    st_ps = ps_rec.tile([DK, DK], f32, name="st_ps",
                                        tag="rec")
                    nc.tensor.matmul(st_ps, kt_all[:, hs], v_bf[:, hs],
                                     start=True, stop=True)
                    nc.vector.tensor_tensor(out=S_st[h], in0=S_st[h],
                                            in1=st_ps, op=ADD)
                    nc.vector.tensor_scalar(out=S_st[h], in0=S_st[h],
                                            scalar1=ebC[:, h:h + 1],
                                            scalar2=float(np.sqrt(DK)),
                                            op0=MUL, op1=MUL)
                    # recast the bf16 shadow for the next chunk's q@S
                    sbf = sb.tile([DK, DK], bf16, name=f"S_bf{h}")
                    nc.scalar.copy(sbf, S_st[h])
                    S_bf[h] = sbf

                def th(f, *a):
                    return lambda: f(*a)

                # order widens the a(h) -> o(h) distance (the mask drain
                # needs ~300ns); the LAST-ITERATION flush slices below index
                # into this list: o0=3, o1=5, o2=6, o3=7 — keep in sync!
                ao_th = [
                    th(emit_a, 0), th(emit_a, 1), th(emit_a, 2),
                    th(emit_o, 0), th(emit_a, 3), th(emit_o, 1),
                    th(emit_o, 2), th(emit_o, 3),
                ]
                return ao_th

            x_bufs = [(x0_bf, x0_f8), (xb1, xb81)]
            rec_thunks = []   # R(ci-1) groups, woven into g-proj(ci)

            for ci in range(NCH + 1):
                last = ci == NCH

                # ---- W(ci-1) slice emitter (interleaved into A's PE stream)
                def wo_slice(ns):
                    if oT_prev is None:
                        return
                    op_ps = ps_big.tile([128, 512], f32, name=f"op{ns}",
                                        tag="big")
                    for h in range(HPG):
                        nc.tensor.matmul(
                            op_ps, oT_prev[:, h * 128:(h + 1) * 128],
                            Wo_t[ns // 2][:, (ns % 2) * 2048 +
                                          h * 512:(ns % 2) * 2048 +
                                          (h + 1) * 512],
                            start=(h == 0), stop=(h == HPG - 1))
                    nc.scalar.copy(
                        out_sb_prev[:, ns * 512:(ns + 1) * 512], op_ps)

                if last:
                    # flush R(NCH-1) with the Wo per-head matmuls interleaved
                    # as each head's oT lands (3 accumulation groups open at
                    # once - ps_big has 3 slots; slice 3 follows after).
                    # Each slice DMAs out the moment its copy lands.
                    ops = {}

                    def wo_h(ns, h):
                        if ns not in ops:
                            ops[ns] = ps_big.tile([128, 512], f32,
                                                  name=f"op{ns}", tag="big")
                        nc.tensor.matmul(
                            ops[ns], oT_prev[:, h * 128:(h + 1) * 128],
                            Wo_t[ns // 2][:, (ns % 2) * 2048 + h * 512:
                                          (ns % 2) * 2048 + (h + 1) * 512],
                            start=(h == 0), stop=(h == HPG - 1))
                        if h == HPG - 1:
                            nc.scalar.copy(
                                out_sb_prev[:, ns * 512:(ns + 1) * 512],
                                ops[ns])
                            nc.sync.dma_start(
                                out=out[t0_prev:t0_prev + 128,
                                        ns * 512:(ns + 1) * 512],
                                in_=out_sb_prev[:, ns * 512:(ns + 1) * 512])

                    for th in rec_thunks[0:4]:   # a0,a1,a2,o0
                        th()
                    for ns in range(3):
                        wo_h(ns, 0)
                    for th in rec_thunks[4:6]:   # a3,o1
                        th()
                    for ns in range(3):
                        wo_h(ns, 1)
                    rec_thunks[6]()              # o2
                    for ns in range(3):
                        wo_h(ns, 2)
                    rec_thunks[7]()              # o3
                    for ns in range(3):
                        wo_h(ns, 3)
                    # slice 3 after the rest (ps_big has 3 slots)
                    op3 = ps_big.tile([128, 512], f32, name="op3", tag="big")
                    for h in range(HPG):
                        nc.tensor.matmul(
                            op3, oT_prev[:, h * 128:(h + 1) * 128],
                            Wo_t[1][:, 2048 + h * 512:2048 + (h + 1) * 512],
                            start=(h == 0), stop=(h == HPG - 1))
                    nc.scalar.copy(out_sb_prev[:, 3 * 512:4 * 512], op3)
                    nc.sync.dma_start(
                        out=out[t0_prev:t0_prev + 128, 3 * 512:4 * 512],
                        in_=out_sb_prev[:, 3 * 512:4 * 512])
                    break

                t0 = ci * C
                xT, xT8 = x_bufs[ci]
                if 1 <= ci and ci + 1 < NCH:
                    # x2/x3 ride the HWDGE queues (FIFO after all setup DMAs,
                    # so they don't steal HBM from the weight stream); x4+ on
                    # gpsimd, naturally gated by pool-slot reuse (the slot's
                    # previous chunk must be consumed first).
                    base = (ci + 1) * D
                    xb = sb.tile([128, D], bf16, name="x_bf", bufs=3)
                    xb8 = sb.tile([128, D], fp8, name="x_f8", bufs=3)
                    if ci + 1 <= 3:
                        qA.dma_start(out=xb, in_=x[:, base:base + D])
                        qB.dma_start(out=xb8, in_=xf8[:, base:base + D])
                    else:
                        nc.gpsimd.dma_start(out=xb, in_=x[:, base:base + D])
                        nc.gpsimd.dma_start(out=xb8, in_=xf8[:, base:base + D])
                    x_bufs.append((xb, xb8))

                out_sb = sb.tile([128, D], bf16, name="out_sb")

                # ---- A stage: projections (R(ci-1) woven into g+q), gate --
                g_ps = project_g(xT8, weave=rec_thunks)
                # s = softplus(-z) = ln(1 + exp(-z)); hi/lo split emitted
                # right away so the b matmuls never wait on the ACT queue
                emz = sb.tile([128, DG], f32, name="emz")
                nc.scalar.activation(emz, g_ps, AF.Exp, scale=-1.0 / 32.0)
                # s in bf16: the cumsum PSUM-accumulates in fp32, so the only
                # loss is per-element bf16 rounding of s — random-walk ~0.9%
                # on extreme decay factors, measured +0.4e-3 end-to-end
                s_bf = sb.tile([128, DG], bf16, name="s_bf")
                nc.scalar.activation(s_bf, emz, AF.Ln, bias=1.0)

                # q projection runs on PE while ACT finishes the gate chain;
                # the R(ci-1) leftovers finish inside it (before wo0, which
                # needs the complete oT). q_ps drains to SBUF immediately
                # (plain copy, no ebq dependency) so its PSUM slot frees
                # before the k projection needs it.
                q_ps = project("q", xT, weave=rec_thunks)
                for th in rec_thunks:   # leftovers
                    th()
                rec_thunks = []
                q_raw = sb.tile([128, DG], f32, name="q_raw")
                nc.scalar.copy(q_raw, q_ps)

                wo_slice(0)

                # in-chunk inclusive cumsum b (tokens on partitions), as two
                # bf16 matmuls over a hi/lo split of s (16-bit mantissa keeps
                # the cumsum accurate at half the fp32 matmul cost)
                b_ps = ps_big.tile([128, DG], f32, name="b_ps", tag="big")
                nc.tensor.matmul(b_ps, M_bf, s_bf, start=True, stop=True)
                # bf16: single-pass PE matmul for the ebC extraction (fp32
                # lhsT = 2-pass + fp32 LDW, ~0.6us/chunk); 0.2% rounding on
                # qt/state-scale is below the bf16 noise already present
                ebq = sb.tile([128, DG], bf16, name="ebq")  # exp(b)/sqrt(DK)
                nc.scalar.activation(ebq, b_ps, AF.Exp, bias=lnq_bias)
                enb = sb.tile([128, DG], bf16, name="enb")  # exp(-b)
                nc.scalar.activation(enb, b_ps, AF.Exp, scale=-1.0)
                qt_all = sb.tile([128, DG], bf16, name="qt_all")
                nc.vector.tensor_tensor(out=qt_all, in0=q_raw, in1=ebq, op=MUL)
                # ONE 3D XBAR transpose: qT_all[p, h, t] = qt_all[t, h*128+p]
                qT_all = sb.tile([128, DG], bf16, name="qT_all")
                nc.sync.dma_start(
                    out=qT_all.rearrange("p (h t) -> p h t", h=HPG),
                    in_=qt_all, transpose=True)

                wo_slice(1)

                # exp(b_C)/sqrt(DK) per head-dim = row C-1 of ebq, extracted
                # with 1-column matmuls woven into the k projection;
                # sqrt(DK) folds into the state scale
                ebC = sb.tile([128, HPG], f32, name="ebC")

                def tc_thunk(h):
                    def go():
                        tc_ps = ps_t.tile([128, 1], f32, name="tp_c", tag="t")
                        nc.tensor.matmul(tc_ps, ebq[:, h * 128:(h + 1) * 128],
                                         e_last, start=True, stop=True)
                        nc.scalar.copy(ebC[:, h:h + 1], tc_ps)
                    return go
                tc_thunks = [tc_thunk(h) for h in range(HPG)]

                k_ps = project("k", xT, weave=tc_thunks, weave_from=6)
                kt_all = sb.tile([128, DG], bf16, name="kt_all")
                nc.vector.tensor_tensor(out=kt_all, in0=k_ps, in1=enb, op=MUL)
                kT_all = sb.tile([128, DG], bf16, name="kT_all")
                nc.sync.dma_start(
                    out=kT_all.rearrange("p (h t) -> p h t", h=HPG),
                    in_=kt_all, transpose=True)

                wo_slice(2)

                # the a/o groups (needing the qT/kT XBAR transposes) weave
                # into g+q(ci+1)
                v_bf = sb.tile([128, DG], bf16, name="v_bf")
                oT_sb = sb.tile([128, DG], bf16, name="oT_sb")
                ao_th = make_rec_thunks(qT_all, kT_all, kt_all, v_bf, ebC,
                                        oT_sb,
                                        update_state=(ci < NCH - 1))
                v_ps = project("v", xT)
                nc.scalar.copy(v_bf, v_ps)

                wo_slice(3)
                if out_sb_prev is not None:
                    nc.sync.dma_start(out=out[t0_prev:t0_prev + 128, :],
                                      in_=out_sb_prev)

                rec_thunks = ao_th

                oT_prev = oT_sb
                out_sb_prev = out_sb
                t0_prev = t0

    _split_waits(nc, mybir)
    return nc


_NC_CACHE = None


def kernel(x, Wq, Wk, Wv, Wg, Wo):
    global _NC_CACHE
    from concourse import bass_utils
    import ml_dtypes

    bf16 = ml_dtypes.bfloat16
    fp8 = ml_dtypes.float8_e4m3fn

    x = np.asarray(x, dtype=np.float32)
    Ws = [np.asarray(w, dtype=np.float32) for w in (Wq, Wk, Wv, Wg)]
    Wo = np.asarray(Wo, dtype=np.float32)

    Mc = np.triu(np.full((C, C), -1.0 / GATE_NORM, np.float32)).astype(bf16)
    Mt = np.triu(np.ones((C, C), np.float32))          # keep j<=i
    el = np.zeros((128, 1), np.float32)
    el[C - 1, 0] = 1.0
    el = el.astype(bf16)

    # x pack: [p, ci, kt, t] so each chunk is one contiguous [128, 2048]
    xp = [x[b].reshape(NCH, C, NKT, 128).transpose(3, 0, 2, 1)
          .reshape(128, NCH * NKT * C) for b in range(B)]
    xbf = [t.astype(bf16) for t in xp]
    xf8 = [t.astype(fp8) for t in xp]

    def pack_w(w):   # [2048, 512] -> [p, kt, m] = [128, NKT*DG]
        return np.ascontiguousarray(
            w.reshape(NKT, 128, DG).transpose(1, 0, 2).reshape(128, NKT * DG))

    in_maps = []
    for c in range(8):
        b, hg = c // 4, c % 4
        cols = slice(hg * DG, (hg + 1) * DG)
        # Wo slice [512, 2048] -> [p, ns, h, m(512)]
        wo_s = Wo[cols, :].reshape(HPG, 128, 4, 512).transpose(
            1, 2, 0, 3).reshape(128, HPG * D)
        m = {"x": xbf[b], "xf8": xf8[b],
             "Wo": np.ascontiguousarray(wo_s).astype(bf16),
             "Mc": Mc, "Mt": Mt, "el": el}
        for name, w in zip(("Wq", "Wk", "Wv"), Ws[:3]):
            m[name] = pack_w(w[:, cols]).astype(bf16)
        m["Wg"] = pack_w(Ws[3][:, cols] * 32.0).astype(fp8)
        in_maps.append(m)

    if _NC_CACHE is None:
        _NC_CACHE = _build()
    r = bass_utils.run_bass_kernel_spmd(_NC_CACHE, in_maps, core_ids=list(range(8)))
    globals()["_LAST_RESULTS"] = r

    res = np.zeros((B, S, D), dtype=np.float32)
    for c in range(8):
        res[c // 4] += r.results[c]["out"].astype(np.float32)
    return res
